# revision 1
# baseline (speedup 1.0000x reference)
"""Trainium2 Bass kernel for shifted-window correlation (27 shifts) + SE gate.

Reference computation (shapes hardcoded; B=1, C=16, W=80, H=96, D=112):
  corr[w,h,d,k] = mean_c x1[c,w,h,d] * x2[c, w+sx, h+sy, d+sz]   (zero-padded)
  s = mean_{w,h,d} corr;  g = sigmoid(w1 @ relu(w0 @ s + b0) + b1)
  out = corr * g

Strategy (8 cores, W sharded 10/core):
  - SBUF partition dim = (c:16, h8:8) where h8 = h // (H/8).
  - x2 loaded ONCE per parity (even/odd d for bf16 4B alignment) as a
    [128, HB+2, Wc+2, D(+2)] tile whose hblk axis carries a 1-row halo:
    row r holds h = h8*HB + (r-1), so all three sy shifts are free-dim
    offsets (the halo rows hold the neighboring h8 block's edge data).
  - Products on DVE (bf16 2x mode); odd-parity dz pairs fused into one
    DVE op via an overlapping strided AP (x1 broadcast along the pair
    axis). Channel reduction on the PE via a fixed block-diagonal
    selection matmul packing (k,h8) into 128/88-row PSUM tiles; the
    A-tile and B-tile accumulation chains are zipped so consecutive
    matmuls alternate PSUM banks and PE column groups.
  - corr stays resident in SBUF (no DRAM spill); ACT drains PSUM->SBUF
    capturing squeeze partials via accum_out.
  - Single squeeze allreduce ([216,2]: early rows reduced on DVE during
    the last row, last row appended post-drain); the gate MLP's first
    layer accumulates both columns as four matmuls into one PSUM tile.
  - DRAM tensors are (row, partition, ...) so every DMA is a contiguous
    burst per partition; ramp-critical loads are split fine and issued
    round-robin over gpsimd/SP/ACT (DGE issue is ~0.6-0.7us each, so a
    single sequencer would gate the ramp); gated writeout from SBUF
    (A rows on ACT + DVE, B rows on DVE 4x), half-row store DMAs issued
    from SP and gpsimd.
"""

import sys
import types

import numpy as np
import ml_dtypes


def _install_ntff_hook_shim():
    """agent image's antenv lacks axon_hooks; needed only for trace=True."""
    if "antenv.axon_hooks" in sys.modules:
        return
    try:
        import antenv
        from trn_agent_boot.trn_boot import _ntff_profile_via_ctypes

        hook = _ntff_profile_via_ctypes("/opt/axon/libaxon_pjrt.so")
        mod = types.ModuleType("antenv.axon_hooks")
        ref = {"h": hook}
        mod.get_axon_ntff_profile_hook = lambda: ref["h"]
        mod.set_axon_ntff_profile_hook = lambda h: ref.__setitem__("h", h)
        sys.modules["antenv.axon_hooks"] = mod
        antenv.axon_hooks = mod
    except Exception:
        pass


_install_ntff_hook_shim()

import concourse.bacc as bacc  # noqa: E402
import concourse.tile as tile  # noqa: E402
import concourse.mybir as mybir  # noqa: E402
from concourse.bass_utils import run_bass_kernel_spmd  # noqa: E402

BF16 = mybir.dt.bfloat16
FP32 = mybir.dt.float32
AF = mybir.ActivationFunctionType
ALU = mybir.AluOpType

N_CORES = 8
C = 16
H8 = 8          # partition sub-dim over h
K = 27
MID = 6

# shifts whose products run on the Pool engine (DVE handles the rest).
# Empty: Pool's software tensor_tensor is ~3us/row-product AND its SBUF
# reads contend with DVE, knocking DVE products out of 2x mode.
POOL_SHIFTS = frozenset()
POOL_STT = False  # walrus rejects scalar_tensor_tensor on Pool


class Cfg:
    def __init__(self, W=80, H=96, D=112):
        assert H % H8 == 0
        self.W, self.H, self.D = W, H, D
        self.Wc = W // N_CORES          # w columns per core
        self.HB = H // H8               # hblk extent (free dim)
        self.De = D + 2                 # odd-copy d extent
        self.FD = self.Wc * D           # flat (w, d) free size per row
        self.slices = [(o, min(o + 512, self.FD))
                       for o in range(0, self.FD, 512)]
        self.n_fs = len(self.slices)
        assert self.HB % 2 == 0 and self.HB >= 2
        self.groups = [(j, 1) for j in range(self.HB)]


# shift order matches reference: k = dx*9 + dy*3 + dz, s* = d*-1
SHIFTS = [(dx - 1, dy - 1, dz - 1)
          for dx in range(3) for dy in range(3) for dz in range(3)]

# PE consumption order: zip the tile-A chain (PSUM banks psA*) with the
# tile-B chain (banks psB*) so consecutive matmuls alternate banks and
# mostly alternate PE column groups, while each bank keeps a single open
# accumulation group at a time. B starts at group 1 to de-align positions.
_A_CHAIN = [4 * g + v for g in range(4) for v in range(4)]
_B_CHAIN = [16 + 4 * g + v for g in (2, 0, 1) for v in range(3 if g == 2 else 4)]
PE_ORDER = []
for _i in range(16):
    PE_ORDER.append(_A_CHAIN[_i])
    if _i < 11:
        PE_ORDER.append(_B_CHAIN[_i])


def _gv_of(k):
    """(is_A, psum column group, selection slice) for shift k."""
    kk = k if k < 16 else k - 16
    return k < 16, kk // 4, kk % 4


def _row_of(k, h8):
    """corr partition row for (k, h8). Tile A: k 0..15, tile B: 16..26."""
    kk = k if k < 16 else k - 16
    base = 0 if k < 16 else 128
    return base + 32 * (kk // 4) + 8 * (kk % 4) + h8


def build_nc(cfg: Cfg):
    nc = bacc.Bacc("TRN2", target_bir_lowering=False, debug=False,
                   num_devices=N_CORES)
    HB, Wc, D, De, FD = cfg.HB, cfg.Wc, cfg.D, cfg.De, cfg.FD

    x1_d = nc.dram_tensor("x1", [HB, 128, Wc, D], BF16, kind="ExternalInput")
    x2e_d = nc.dram_tensor("x2e", [HB + 2, 128, Wc + 2, D], BF16,
                           kind="ExternalInput")
    x2o_d = nc.dram_tensor("x2o", [HB + 2, 128, Wc + 2, De], BF16,
                           kind="ExternalInput")
    sel_d = nc.dram_tensor("selmats", [128, 128], BF16, kind="ExternalInput")
    w0a_d = nc.dram_tensor("w0a", [128, MID], FP32, kind="ExternalInput")
    w0b_d = nc.dram_tensor("w0b", [88, MID], FP32, kind="ExternalInput")
    w1a_d = nc.dram_tensor("w1ra", [MID, 128], FP32, kind="ExternalInput")
    w1b_d = nc.dram_tensor("w1rb", [MID, 88], FP32, kind="ExternalInput")
    b0_d = nc.dram_tensor("b0c", [MID, 1], FP32, kind="ExternalInput")
    b1a_d = nc.dram_tensor("b1ra", [128, 1], FP32, kind="ExternalInput")
    b1b_d = nc.dram_tensor("b1rb", [88, 1], FP32, kind="ExternalInput")
    out_d = nc.dram_tensor("out", [HB, 216, FD], BF16, kind="ExternalOutput")

    n_drain = HB * cfg.n_fs
    wh = (Wc + 2) // 2 or 1         # x2 w-half for split loads
    xh = max(Wc // 2, 1)            # x1 w-half

    with tile.TileContext(nc) as tc:
        with (
            tc.tile_pool(name="const", bufs=1) as cpool,
            tc.tile_pool(name="ps", bufs=1, space="PSUM") as ps,
            tc.tile_pool(name="dram", bufs=1, space="DRAM") as dram,
            tc.tile_pool(name="pp", bufs=5) as ppool,
            tc.tile_pool(name="qq", bufs=2) as qpool,
            tc.tile_pool(name="stage", bufs=2) as spool,
        ):
            # resident tiles
            x1t = cpool.tile([128, HB, Wc, D], BF16)
            x2e_t = cpool.tile([128, HB + 2, Wc + 2, D], BF16)
            x2o_t = cpool.tile([128, HB + 2, Wc + 2, De], BF16)
            corrA = cpool.tile([128, HB, FD], BF16)
            corrB = cpool.tile([88, HB, FD], BF16)
            selt = cpool.tile([128, 128], BF16)
            w0at = cpool.tile([128, MID], FP32)
            w0bt = cpool.tile([88, MID], FP32)
            w1at = cpool.tile([MID, 128], FP32)
            w1bt = cpool.tile([MID, 88], FP32)
            b0t = cpool.tile([MID, 1], FP32)
            b1at = cpool.tile([128, 1], FP32)
            b1bt = cpool.tile([88, 1], FP32)
            accA = cpool.tile([128, n_drain], FP32)
            accB = cpool.tile([88, n_drain], FP32)

            rr_state = [0]
            rr_engs = None

            def _eng(eng):
                if eng is not None:
                    return eng
                if rr_engs is None:
                    return nc.sync
                e = rr_engs[rr_state[0] % len(rr_engs)]
                rr_state[0] += 1
                return e

            def load_x1_row(r, parts=2, eng=None):
                cuts = [round(i * Wc / parts) for i in range(parts + 1)]
                for a, b in zip(cuts, cuts[1:]):
                    if a < b:
                        _eng(eng).dma_start(x1t[:, r, a:b, :],
                                            x1_d[r, :, a:b, :])

            def load_x2_row(r, parts=2, eng=None):
                cuts = [round(i * (Wc + 2) / parts) for i in range(parts + 1)]
                for t, d in ((x2o_t, x2o_d), (x2e_t, x2e_d)):
                    for a, b in zip(cuts, cuts[1:]):
                        if a < b:
                            _eng(eng).dma_start(t[:, r, a:b, :],
                                                d[r, :, a:b, :])

            # priority-ordered input loads: row-0 products touch x2 rows
            # 0-2 (both parities) + x1 row 0 — those go first at finest
            # granularity, issued from gpsimd whose DGE dispatch is ~25ns
            # (SP pays 565ns per issue, which would gate the ramp).
            rr_engs = [nc.gpsimd, nc.sync, nc.scalar]
            qc = [round(i * (Wc + 2) / 4) for i in range(5)]
            xc = [round(i * Wc / 4) for i in range(5)]
            for qi in range(4):
                if xc[qi] < xc[qi + 1]:
                    _eng(None).dma_start(
                        x1t[:, 0, xc[qi]:xc[qi + 1], :],
                        x1_d[0, :, xc[qi]:xc[qi + 1], :])
                for t, d in ((x2o_t, x2o_d), (x2e_t, x2e_d)):
                    if qc[qi] < qc[qi + 1]:
                        _eng(None).dma_start(t[:, 0, qc[qi]:qc[qi + 1], :],
                                             d[0, :, qc[qi]:qc[qi + 1], :])
            _eng(None).dma_start(selt[:], sel_d[:])
            for r in range(1, min(3, HB + 2)):
                load_x2_row(r, parts=4)
            if HB > 1:
                load_x1_row(1, parts=2)
            for r in range(3, min(4, HB + 2)):
                load_x2_row(r, parts=2)
            rr_engs = None
            nc.sync.dma_start(w0at[:], w0a_d[:])
            nc.sync.dma_start(w0bt[:], w0b_d[:])
            nc.sync.dma_start(w1at[:], w1a_d[:])
            nc.sync.dma_start(w1bt[:], w1b_d[:])
            nc.sync.dma_start(b0t[:], b0_d[:])
            nc.sync.dma_start(b1at[:], b1a_d[:])
            nc.sync.dma_start(b1bt[:], b1b_d[:])

            # Warm-up collective: absorbs cross-core launch skew and CC
            # firmware setup so the real allreduces only pay marginal latency.
            warm_in = dram.tile([MID, 1], FP32)
            warm_out = dram.tile([MID, 1], FP32)
            nc.sync.dma_start(warm_in[:], b0_d[:])
            nc.gpsimd.collective_compute(
                "AllReduce", ALU.add,
                replica_groups=[list(range(N_CORES))],
                ins=[warm_in[:].opt()],
                outs=[warm_out[:].opt()],
            )

            # remaining loads, interleaved in order of first use
            nx1 = 2
            for r in range(4, HB + 2, 2):
                while nx1 < min(r - 1, HB):
                    load_x1_row(nx1)
                    nx1 += 1
                load_x2_row(r)
                if r + 1 < HB + 2:
                    load_x2_row(r + 1)
            while nx1 < HB:
                load_x1_row(nx1)
                nx1 += 1

            cc_in = dram.tile([216, 1 + cfg.n_fs], FP32)
            cc_out = dram.tile([216, 1 + cfg.n_fs], FP32)

            a_tot = {g: 4 for g in range(4)}
            b_tot = {0: 4, 1: 4, 2: 3}

            pe_pos = {k: i for i, k in enumerate(PE_ORDER)}
            # fuseable odd-parity dz-pairs (both members close in PE order)
            pair_bases = [kb for kb in range(0, K, 3)
                          if kb != 15 and abs(pe_pos[kb] - pe_pos[kb + 2]) <= 6]
            paired = {kb for kb in pair_bases} | {kb + 2 for kb in pair_bases}
            units = [("pair", kb) for kb in pair_bases]
            units += [("single", k) for k in range(K) if k not in paired]
            units.sort(key=lambda u: pe_pos[u[1]] if u[0] == "single"
                       else min(pe_pos[u[1]], pe_pos[u[1] + 2]))

            def emit_products(j, mid_hook=None):
                """products for all 27 shifts of row j. Odd-parity dz-pairs
                are fused into one DVE op via an overlapping strided AP."""
                prods = {}
                n_emitted = 0
                for kind, k in units:
                    if n_emitted >= 10 and mid_hook is not None:
                        mid_hook()
                        mid_hook = None
                    sx, sy, sz = SHIFTS[k]
                    pt = ppool.tile([128, 2, FD], BF16, tag="P", bufs=5)
                    if kind == "pair":
                        base = x2o_t[:, 1 + j + sy, 1 + sx:1 + sx + Wc, 0:De]
                        u = base.unsqueeze(1)
                        ap2 = [list(p) for p in u.ap]
                        ap2[1] = [2, 2]
                        ap2[3] = [1, D]
                        from concourse.ap import AP as _AP
                        srcv = _AP(tensor=u.tensor, offset=u.offset, ap=ap2)
                        x1b = x1t[:, j].unsqueeze(1).broadcast_to(
                            [128, 2, Wc, D])
                        dst = pt[:, 0:2].rearrange("p r (w d) -> p r w d", d=D)
                        nc.vector.tensor_tensor(dst, x1b, srcv, ALU.mult)
                        prods[k] = (pt, 0)
                        prods[k + 2] = (pt, 1)
                        n_emitted += 2
                    else:
                        if sz == 0:
                            srcv = x2e_t[:, 1 + j + sy, 1 + sx:1 + sx + Wc,
                                         0:D]
                        else:
                            doff = sz + 1
                            srcv = x2o_t[:, 1 + j + sy, 1 + sx:1 + sx + Wc,
                                         doff:doff + D]
                        dst = pt[:, 0].rearrange("p (w d) -> p w d", d=D)
                        nc.vector.tensor_tensor(dst, x1t[:, j, :, :], srcv,
                                                ALU.mult)
                        prods[k] = (pt, 0)
                        n_emitted += 1
                return prods

            def emit_reduce_row(j, prods):
                """PE reduction + ACT drains for row j."""
                psA = [ps.tile([128, hi - lo], FP32, tag=f"psA{i}",
                               name=f"psA{i}", padded_shape=[128, 512])
                       for i, (lo, hi) in enumerate(cfg.slices)]
                psB = [ps.tile([128, hi - lo], FP32, tag=f"psB{i}",
                               name=f"psB{i}", padded_shape=[128, 512])
                       for i, (lo, hi) in enumerate(cfg.slices)]
                seen = {}
                for k in PE_ORDER:
                    is_a, g, v = _gv_of(k)
                    tot = a_tot[g] if is_a else b_tot[g]
                    cnt = seen.get((is_a, g), 0)
                    seen[(is_a, g)] = cnt + 1
                    pst = psA if is_a else psB
                    for i, (lo, hi) in enumerate(cfg.slices):
                        pk, idx = prods[k]
                        nc.tensor.matmul(
                            pst[i][32 * g:32 * g + 32, :],
                            selt[:, 32 * v:32 * v + 32],
                            pk[:, idx, lo:hi],
                            start=(cnt == 0), stop=(cnt == tot - 1),
                            tile_position=(0, 32 * g),
                        )
                    if k == _B_CHAIN[-1]:
                        # B chain done while A's tail streams: drain B now
                        for i, (lo, hi) in enumerate(cfg.slices):
                            di = j * cfg.n_fs + i
                            nc.scalar.activation(
                                corrB[:, j, lo:hi], psB[i][0:88, :], AF.Copy,
                                accum_out=accB[:, di:di + 1])
                for i, (lo, hi) in enumerate(cfg.slices):
                    di = j * cfg.n_fs + i
                    nc.scalar.activation(
                        corrA[:, j, lo:hi], psA[i][:], AF.Copy,
                        accum_out=accA[:, di:di + 1])

            early_cols = (HB - 1) * cfg.n_fs
            pAe = cpool.tile([128, 1], FP32)
            pBe = cpool.tile([88, 1], FP32)

            def early_squeeze():
                # rows 0..HB-2 partial sums, reduced on DVE between last-row
                # products and shipped early so the post-drain cc path only
                # handles the last row's 3 columns.
                nc.vector.tensor_reduce(pAe[:], accA[:, 0:early_cols],
                                        mybir.AxisListType.X, ALU.add)
                nc.vector.tensor_reduce(pBe[:], accB[:, 0:early_cols],
                                        mybir.AxisListType.X, ALU.add)
                nc.gpsimd.dma_start(cc_in[0:128, 0:1], pAe[:])
                nc.gpsimd.dma_start(cc_in[128:216, 0:1], pBe[:])

            for gi, (j0, nr) in enumerate(cfg.groups):
                hook = early_squeeze if gi == len(cfg.groups) - 1 else None
                prods = emit_products(j0, mid_hook=hook)
                emit_reduce_row(j0, prods)

            # ---- last-row raw accum cols straight into the allreduce ----
            nlate = n_drain - early_cols
            nc.scalar.dma_start(cc_in[0:128, 1:1 + nlate],
                                accA[:, early_cols:n_drain])
            nc.scalar.dma_start(cc_in[128:216, 1:1 + nlate],
                                accB[:, early_cols:n_drain])
            nc.gpsimd.collective_compute(
                "AllReduce", ALU.add,
                replica_groups=[list(range(N_CORES))],
                ins=[cc_in[:].opt()],
                outs=[cc_out[:].opt()],
            )
            pAg = cpool.tile([128, 1 + cfg.n_fs], FP32)
            pBg = cpool.tile([88, 1 + cfg.n_fs], FP32)
            nc.gpsimd.dma_start(pAg[:], cc_out[0:128, :])
            nc.gpsimd.dma_start(pBg[:], cc_out[128:216, :])

            hps = ps.tile([MID, 1], FP32, tag="psA0", padded_shape=[128, 512])
            for c in range(1 + nlate):
                nc.tensor.matmul(hps[:], w0at[:], pAg[:, c:c + 1],
                                 start=(c == 0), stop=False)
                nc.tensor.matmul(hps[:], w0bt[:], pBg[:, c:c + 1],
                                 start=False, stop=(c == nlate))
            hvec = cpool.tile([MID, 1], FP32)
            nc.scalar.activation(hvec[:], hps[:], AF.Relu, bias=b0t[:],
                                 scale=1.0)
            gpsA = ps.tile([128, 1], FP32, tag="psA1", padded_shape=[128, 512])
            gpsB = ps.tile([88, 1], FP32, tag="psA2", padded_shape=[128, 512])
            nc.tensor.matmul(gpsA[:], w1at[:], hvec[:], start=True, stop=True)
            nc.tensor.matmul(gpsB[:], w1bt[:], hvec[:], start=True, stop=True)
            gA = cpool.tile([128, 1], FP32)
            gB = cpool.tile([88, 1], FP32)
            nc.scalar.activation(gA[:], gpsA[:], AF.Sigmoid, bias=b1at[:],
                                 scale=1.0)
            nc.scalar.activation(gB[:], gpsB[:], AF.Sigmoid, bias=b1bt[:],
                                 scale=1.0)

            # ---- gated writeout from SBUF (A on ACT, B on DVE 4x).
            # Half-row output DMAs, all issued from the idle Pool sequencer
            # (cheapest DGE dispatch) to keep ACT/SP free for gating. ----
            fh = (FD // 2 + 1) & ~1 if FD > 2 else FD
            act_rows = (HB * 2) // 3
            for j in range(HB):
                stA = spool.tile([128, FD], BF16, tag="gsA", bufs=6)
                if j < act_rows:
                    nc.scalar.mul(stA[:], corrA[:, j, :], gA[:])
                else:
                    nc.vector.tensor_scalar(stA[:], corrA[:, j, :], gA[:],
                                            None, ALU.mult)
                stB = spool.tile([88, FD], BF16, tag="gsB", bufs=6)
                nc.vector.tensor_scalar(stB[:], corrB[:, j, :], gB[:],
                                        None, ALU.mult)
                nc.sync.dma_start(out_d[j, 0:128, 0:fh], stA[:, 0:fh])
                nc.sync.dma_start(out_d[j, 0:128, fh:FD], stA[:, fh:FD])
                nc.gpsimd.dma_start(out_d[j, 128:216, 0:fh], stB[:, 0:fh])
                nc.gpsimd.dma_start(out_d[j, 128:216, fh:FD], stB[:, fh:FD])

    nc.compile()
    return nc


# ---------------- host-side prep / assembly ----------------

def make_gate_consts(w0, b0, w1, b1, cfg: Cfg):
    norm = 1.0 / (cfg.W * cfg.H * cfg.D)
    sel = np.zeros((128, 128), dtype=np.float32)
    for v in range(4):
        for c in range(C):
            for h8 in range(H8):
                sel[c * H8 + h8, 32 * v + 8 * v + h8] = 1.0 / 16
    w0 = np.asarray(w0, dtype=np.float32)
    w1 = np.asarray(w1, dtype=np.float32)
    b1 = np.asarray(b1, dtype=np.float32)
    w0a = np.zeros((128, MID), dtype=np.float32)
    w0b = np.zeros((88, MID), dtype=np.float32)
    w1ra = np.zeros((MID, 128), dtype=np.float32)
    w1rb = np.zeros((MID, 88), dtype=np.float32)
    b1ra = np.zeros((128, 1), dtype=np.float32)
    b1rb = np.zeros((88, 1), dtype=np.float32)
    for k in range(K):
        for h8 in range(H8):
            r = _row_of(k, h8)
            if k < 16:
                w0a[r, :] = w0[:, k] * norm
                w1ra[:, r] = w1[k, :]
                b1ra[r, 0] = b1[k]
            else:
                w0b[r - 128, :] = w0[:, k] * norm
                w1rb[:, r - 128] = w1[k, :]
                b1rb[r - 128, 0] = b1[k]
    return {
        "selmats": sel.astype(ml_dtypes.bfloat16),
        "w0a": w0a, "w0b": w0b, "w1ra": w1ra, "w1rb": w1rb,
        "b0c": np.asarray(b0, dtype=np.float32).reshape(MID, 1),
        "b1ra": b1ra, "b1rb": b1rb,
    }


def _fold(a, HB):
    # [C, w, H, D'] -> [(c h8), hblk, w, d]
    Cc, ww, hh, dd = a.shape
    a = a.reshape(Cc, ww, H8, HB, dd)
    a = np.ascontiguousarray(a.transpose(0, 2, 3, 1, 4))
    return a.reshape(C * H8, HB, ww, dd)


def make_inputs_per_core(x_1, x_2, w0, b0, w1, b1, cfg: Cfg):
    """x_1/x_2: [1, C, W, H, D] float32 -> list of per-core input dicts."""
    W, H, D, De = cfg.W, cfg.H, cfg.D, cfg.De
    Wc, HB = cfg.Wc, cfg.HB
    x1 = np.asarray(x_1)[0].astype(ml_dtypes.bfloat16)      # [C, W, H, D]
    x2 = np.asarray(x_2)[0].astype(ml_dtypes.bfloat16)
    # padded x2: w +-1, h +-1, d in [-1, D+1)
    x2p = np.zeros((C, W + 2, H + 2, D + 2), dtype=ml_dtypes.bfloat16)
    x2p[:, 1:W + 1, 1:H + 1, 1:D + 1] = x2
    # hblk-extended h indices: row r of (h8) block = x2p h-index h8*HB + r,
    # covering h = h8*HB - 1 .. (h8+1)*HB (1-voxel halo on both sides)
    hidx = (np.arange(H8) * HB)[:, None] + np.arange(HB + 2)  # [H8, HB+2]

    consts = make_gate_consts(w0, b0, w1, b1, cfg)
    in_maps = []
    for ci in range(N_CORES):
        ws = ci * Wc
        m = dict(consts)
        m["x1"] = np.ascontiguousarray(
            _fold(x1[:, ws:ws + Wc, :, :], HB).transpose(1, 0, 2, 3))
        blk = x2p[:, ws:ws + Wc + 2, :, :]                  # [C, Wc+2, H+2, De]
        ee = blk[:, :, hidx, 1:1 + D]                       # [C, Wc+2, H8, HB+2, D]
        oo = blk[:, :, hidx, 0:De]
        m["x2e"] = np.ascontiguousarray(
            ee.transpose(3, 0, 2, 1, 4)).reshape(HB + 2, 128, Wc + 2, D)
        m["x2o"] = np.ascontiguousarray(
            oo.transpose(3, 0, 2, 1, 4)).reshape(HB + 2, 128, Wc + 2, De)
        in_maps.append(m)
    return in_maps


def assemble_output(results, cfg: Cfg):
    W, H, D = cfg.W, cfg.H, cfg.D
    Wc, HB = cfg.Wc, cfg.HB
    rows = np.empty((K, H8), dtype=np.int64)
    for k in range(K):
        for h8 in range(H8):
            rows[k, h8] = _row_of(k, h8)
    out = np.empty((W, H, D, K), dtype=np.float32)
    for ci, r in enumerate(results):
        o = np.asarray(r["out"]).reshape(HB, 216, Wc, D)
        o = o.transpose(1, 0, 2, 3)
        core = o[rows]                        # [K, H8, HB, Wc, D]
        core = core.transpose(3, 1, 2, 4, 0)  # [Wc, H8, HB, D, K]
        out[ci * Wc:(ci + 1) * Wc] = core.reshape(Wc, H, D, K)
    return out[None]


_CACHE = {}
TRACE = False           # test harness can set kernel.TRACE = True


def kernel(x_1, x_2, w0, b0, w1, b1):
    cfg = Cfg()
    if "nc" not in _CACHE:
        _CACHE["nc"] = build_nc(cfg)
    nc = _CACHE["nc"]
    in_maps = make_inputs_per_core(x_1, x_2, w0, b0, w1, b1, cfg)
    last_exc = None
    for _attempt in range(3):
        try:
            res = run_bass_kernel_spmd(nc, in_maps,
                                       core_ids=list(range(N_CORES)),
                                       trace=TRACE)
            break
        except Exception as e:  # transient NRT device errors: retry
            last_exc = e
    else:
        raise last_exc
    _CACHE["last_res"] = res
    return assemble_output(res.results, cfg)



# revision 2
# speedup vs baseline: 1.1145x; 1.1145x over previous
"""Trainium2 Bass kernel for shifted-window correlation (27 shifts) + SE gate.

Reference computation (shapes hardcoded; B=1, C=16, W=80, H=96, D=112):
  corr[w,h,d,k] = mean_c x1[c,w,h,d] * x2[c, w+sx, h+sy, d+sz]   (zero-padded)
  s = mean_{w,h,d} corr;  g = sigmoid(w1 @ relu(w0 @ s + b0) + b1)
  out = corr * g

Strategy (8 cores, W sharded 10/core), v2:
  - SBUF partition dim = (c:16, h8:8) where h8 = h // (H/8).
  - ONE resident x2 tile per core ([128, HB+2, Wc+2, De], De=D+2) whose
    hblk axis carries a 1-row halo; all 27 shifts are free-dim offsets.
    (bf16 DVE speed is offset-alignment-independent — measured — so no
    even/odd d copies.)
  - Products on DVE (bf16 2x mode, ~0.52ns/elem): one fused op per
    (sx,sy) computing all three dz via an overlapping unit-stride AP
    (3 free dims is the TensorTensor ISA limit) -> 9 ops/row.
  - Channel reduction on PE via a block-diagonal selection matmul
    packing (k,h8) into 128/88-row PSUM tiles; natural k order so
    consecutive matmuls cycle the 3 slice banks; A banks drain on ACT
    (capturing squeeze partials via accum_out) right after k=15 so the
    next row never stalls on PSUM.
  - Squeeze allreduce split in two: CC1 ([216,1], rows 0..HB-2) fires
    mid-way through the last row's products and absorbs cross-core
    skew; CC2 ([216,1], last row) right after the final drain only
    pays warm-latency (~9us). Gate MLP accumulates both columns.
  - relu on DVE + sigmoid table preloaded at ramp so the tail has no
    ACT_TABLE_LOAD.
  - Gated writeout overlaps: ACT gates 1/3 of A rows + DVE the rest;
    full-row output DMAs on HWDGE queues only (sync + a few on scalar)
    -- gpsimd SWDGE would starve against DVE perf-mode gating ops.
"""

import sys
import types

import numpy as np
import ml_dtypes


def _install_ntff_hook_shim():
    """agent image's antenv lacks axon_hooks; needed only for trace=True."""
    if "antenv.axon_hooks" in sys.modules:
        return
    try:
        import antenv
        from trn_agent_boot.trn_boot import _ntff_profile_via_ctypes

        hook = _ntff_profile_via_ctypes("/opt/axon/libaxon_pjrt.so")
        mod = types.ModuleType("antenv.axon_hooks")
        ref = {"h": hook}
        mod.get_axon_ntff_profile_hook = lambda: ref["h"]
        mod.set_axon_ntff_profile_hook = lambda h: ref.__setitem__("h", h)
        sys.modules["antenv.axon_hooks"] = mod
        antenv.axon_hooks = mod
    except Exception:
        pass


_install_ntff_hook_shim()

import concourse.bacc as bacc  # noqa: E402
import concourse.tile as tile  # noqa: E402
import concourse.mybir as mybir  # noqa: E402
from concourse.ap import AP as _AP  # noqa: E402
from concourse.bass_utils import run_bass_kernel_spmd  # noqa: E402

BF16 = mybir.dt.bfloat16
FP32 = mybir.dt.float32
AF = mybir.ActivationFunctionType
ALU = mybir.AluOpType

N_CORES = 8
C = 16
H8 = 8          # partition sub-dim over h
K = 27
MID = 6


class Cfg:
    def __init__(self, W=80, H=96, D=112):
        assert H % H8 == 0
        self.W, self.H, self.D = W, H, D
        self.Wc = W // N_CORES          # w columns per core
        self.HB = H // H8               # hblk extent (free dim)
        self.De = D + 2                 # padded d extent
        self.FD = self.Wc * D           # flat (w, d) free size per row
        self.slices = [(o, min(o + 512, self.FD))
                       for o in range(0, self.FD, 512)]
        self.n_fs = len(self.slices)


# shift order matches reference: k = dx*9 + dy*3 + dz, s* = d*-1
SHIFTS = [(dx - 1, dy - 1, dz - 1)
          for dx in range(3) for dy in range(3) for dz in range(3)]


def _gv_of(k):
    """(is_A, psum column group, selection slice) for shift k."""
    kk = k if k < 16 else k - 16
    return k < 16, kk // 4, kk % 4


def _row_of(k, h8):
    """corr partition row for (k, h8). Tile A: k 0..15, tile B: 16..26."""
    kk = k if k < 16 else k - 16
    base = 0 if k < 16 else 128
    return base + 32 * (kk // 4) + 8 * (kk % 4) + h8


def build_nc(cfg: Cfg):
    nc = bacc.Bacc("TRN2", target_bir_lowering=False, debug=False,
                   num_devices=N_CORES)
    HB, Wc, D, De, FD = cfg.HB, cfg.Wc, cfg.D, cfg.De, cfg.FD

    x1_d = nc.dram_tensor("x1", [HB, 128, Wc, D], BF16, kind="ExternalInput")
    x2_d = nc.dram_tensor("x2", [HB + 2, 128, Wc + 2, De], BF16,
                          kind="ExternalInput")
    sel_d = nc.dram_tensor("selmats", [128, 128], BF16, kind="ExternalInput")
    w0a_d = nc.dram_tensor("w0a", [128, MID], FP32, kind="ExternalInput")
    w0b_d = nc.dram_tensor("w0b", [88, MID], FP32, kind="ExternalInput")
    w1a_d = nc.dram_tensor("w1ra", [MID, 128], FP32, kind="ExternalInput")
    w1b_d = nc.dram_tensor("w1rb", [MID, 88], FP32, kind="ExternalInput")
    b0_d = nc.dram_tensor("b0c", [MID, 1], FP32, kind="ExternalInput")
    b1a_d = nc.dram_tensor("b1ra", [128, 1], FP32, kind="ExternalInput")
    b1b_d = nc.dram_tensor("b1rb", [88, 1], FP32, kind="ExternalInput")
    out_d = nc.dram_tensor("out", [HB, 216, FD], BF16, kind="ExternalOutput")

    n_drain = HB * cfg.n_fs
    early = (HB - 1) * cfg.n_fs     # accum cols covered by CC1

    with tile.TileContext(nc) as tc:
        with (
            tc.tile_pool(name="const", bufs=1) as cpool,
            tc.tile_pool(name="ps", bufs=1, space="PSUM") as ps,
            tc.tile_pool(name="dram", bufs=1, space="DRAM") as dram,
            tc.tile_pool(name="pp", bufs=6) as ppool,
            tc.tile_pool(name="stage", bufs=4) as spool,
        ):
            # resident tiles
            x1t = cpool.tile([128, HB, Wc, D], BF16)
            x2t = cpool.tile([128, HB + 2, Wc + 2, De], BF16)
            corrA = cpool.tile([128, HB, FD], BF16)
            corrB = cpool.tile([88, HB, FD], BF16)
            selt = cpool.tile([128, 128], BF16)
            w0at = cpool.tile([128, MID], FP32)
            w0bt = cpool.tile([88, MID], FP32)
            w1at = cpool.tile([MID, 128], FP32)
            w1bt = cpool.tile([MID, 88], FP32)
            b0t = cpool.tile([MID, 1], FP32)
            b1at = cpool.tile([128, 1], FP32)
            b1bt = cpool.tile([88, 1], FP32)
            accA = cpool.tile([128, n_drain], FP32)
            accB = cpool.tile([88, n_drain], FP32)
            pA1 = cpool.tile([128, 1], FP32)
            pB1 = cpool.tile([88, 1], FP32)
            pA2 = cpool.tile([128, 1], FP32)
            pB2 = cpool.tile([88, 1], FP32)
            pAg = cpool.tile([128, 2], FP32)
            pBg = cpool.tile([88, 2], FP32)
            dumin = cpool.tile([1, 2], FP32)
            dumout = cpool.tile([1, 2], FP32)

            rr_state = [0]
            rr_engs = [nc.gpsimd, nc.sync, nc.scalar]

            def _rr():
                e = rr_engs[rr_state[0] % len(rr_engs)]
                rr_state[0] += 1
                return e

            def load_x1_row(r, parts=1, eng=None):
                cuts = [round(i * Wc / parts) for i in range(parts + 1)]
                for a, b in zip(cuts, cuts[1:]):
                    if a < b:
                        (eng or _rr()).dma_start(x1t[:, r, a:b, :],
                                                 x1_d[r, :, a:b, :])

            def load_x2_row(r, parts=1, eng=None):
                cuts = [round(i * (Wc + 2) / parts) for i in range(parts + 1)]
                for a, b in zip(cuts, cuts[1:]):
                    if a < b:
                        (eng or _rr()).dma_start(x2t[:, r, a:b, :],
                                                 x2_d[r, :, a:b, :])

            # priority ramp: first product (dx=-1,dy=-1) needs x2 row 0 +
            # x1 row 0; then x2 rows 1, 2. Fine-grained, round-robin over
            # the three DGE-capable sequencers.
            load_x1_row(0, parts=4)
            load_x2_row(0, parts=4)
            load_x2_row(1, parts=4)
            _rr().dma_start(selt[:], sel_d[:])
            load_x2_row(2, parts=2)
            if HB > 1:
                load_x1_row(1, parts=2)
            load_x2_row(3, parts=2)
            nc.sync.dma_start(w0at[:], w0a_d[:])
            nc.sync.dma_start(w0bt[:], w0b_d[:])
            nc.sync.dma_start(w1at[:], w1a_d[:])
            nc.sync.dma_start(w1bt[:], w1b_d[:])
            nc.sync.dma_start(b0t[:], b0_d[:])
            nc.sync.dma_start(b1at[:], b1a_d[:])
            nc.sync.dma_start(b1bt[:], b1b_d[:])

            # preload the sigmoid ACT table so the tail pays no table swap
            nc.gpsimd.memset(dumin[:], 0.0)
            nc.scalar.activation(dumout[:], dumin[:], AF.Sigmoid)

            # Warm-up collective: absorbs cross-core launch skew and CC
            # firmware setup so the real allreduces only pay marginal cost.
            warm_in = dram.tile([MID, 1], FP32)
            warm_out = dram.tile([MID, 1], FP32)
            nc.sync.dma_start(warm_in[:], b0_d[:])
            nc.gpsimd.collective_compute(
                "AllReduce", ALU.add,
                replica_groups=[list(range(N_CORES))],
                ins=[warm_in[:].opt()],
                outs=[warm_out[:].opt()],
            )

            # remaining loads, ~one row ahead of first use
            nx1 = 2
            for r in range(4, HB + 2):
                if nx1 < min(r - 1, HB):
                    load_x1_row(nx1)
                    nx1 += 1
                load_x2_row(r)
            while nx1 < HB:
                load_x1_row(nx1)
                nx1 += 1

            cc1_in = dram.tile([216, 1], FP32)
            cc1_out = dram.tile([216, 1], FP32)
            cc2_in = dram.tile([216, 1], FP32)
            cc2_out = dram.tile([216, 1], FP32)

            a_tot = {g: 4 for g in range(4)}
            b_tot = {0: 4, 1: 4, 2: 3}

            def emit_products(j, mid_hook=None):
                """9 fused (sx,sy) product ops for row j; each computes all
                three dz shifts via an overlapping unit-stride AP."""
                prods = {}
                for t, (dx, dy) in enumerate(
                        (a, b) for a in range(3) for b in range(3)):
                    if t == 4 and mid_hook is not None:
                        mid_hook()
                    pt = ppool.tile([128, 3, FD], BF16, tag="P", bufs=6)
                    base = x2t[:, j + dy, dx:dx + Wc, 0:De]
                    u = base.unsqueeze(1)
                    ap2 = [list(p) for p in u.ap]
                    ap2[1] = [1, 3]
                    ap2[3] = [1, D]
                    srcv = _AP(tensor=u.tensor, offset=u.offset, ap=ap2)
                    x1b = x1t[:, j].unsqueeze(1).broadcast_to([128, 3, Wc, D])
                    dst = pt[:].rearrange("p r (w d) -> p r w d", d=D)
                    nc.vector.tensor_tensor(dst, x1b, srcv, ALU.mult)
                    kb = 9 * dx + 3 * dy
                    for dz in range(3):
                        prods[kb + dz] = (pt, dz)
                return prods

            def emit_reduce_row(j, prods):
                """PE reduction + ACT drains for row j, natural k order."""
                psA = [ps.tile([128, hi - lo], FP32, tag=f"psA{i}",
                               name=f"psA{i}", padded_shape=[128, 512])
                       for i, (lo, hi) in enumerate(cfg.slices)]
                psB = [ps.tile([128, hi - lo], FP32, tag=f"psB{i}",
                               name=f"psB{i}", padded_shape=[128, 512])
                       for i, (lo, hi) in enumerate(cfg.slices)]
                seen = {}
                for k in range(K):
                    is_a, g, v = _gv_of(k)
                    tot = a_tot[g] if is_a else b_tot[g]
                    cnt = seen.get((is_a, g), 0)
                    seen[(is_a, g)] = cnt + 1
                    pst = psA if is_a else psB
                    pk, idx = prods[k]
                    for i, (lo, hi) in enumerate(cfg.slices):
                        nc.tensor.matmul(
                            pst[i][32 * g:32 * g + 32, :],
                            selt[:, 32 * v:32 * v + 32],
                            pk[:, idx, lo:hi],
                            start=(cnt == 0), stop=(cnt == tot - 1),
                            tile_position=(0, 32 * g),
                        )
                    if k == 15:
                        # A chains complete: drain now so the next row's
                        # A matmuls never wait on these banks.
                        for i, (lo, hi) in enumerate(cfg.slices):
                            di = j * cfg.n_fs + i
                            nc.scalar.activation(
                                corrA[:, j, lo:hi], psA[i][:], AF.Copy,
                                accum_out=accA[:, di:di + 1])
                for i, (lo, hi) in enumerate(cfg.slices):
                    di = j * cfg.n_fs + i
                    nc.scalar.activation(
                        corrB[:, j, lo:hi], psB[i][0:88, :], AF.Copy,
                        accum_out=accB[:, di:di + 1])

            def cc1_hook():
                # rows 0..HB-2 partial sums: reduce on DVE, allreduce while
                # the last row's products still run (absorbs cross-core skew)
                nc.vector.tensor_reduce(pA1[:], accA[:, 0:early],
                                        mybir.AxisListType.X, ALU.add)
                nc.vector.tensor_reduce(pB1[:], accB[:, 0:early],
                                        mybir.AxisListType.X, ALU.add)
                nc.gpsimd.dma_start(cc1_in[0:128, :], pA1[:])
                nc.gpsimd.dma_start(cc1_in[128:216, :], pB1[:])
                nc.gpsimd.collective_compute(
                    "AllReduce", ALU.add,
                    replica_groups=[list(range(N_CORES))],
                    ins=[cc1_in[:].opt()],
                    outs=[cc1_out[:].opt()],
                )
                nc.gpsimd.dma_start(pAg[:, 0:1], cc1_out[0:128, :])
                nc.gpsimd.dma_start(pBg[:, 0:1], cc1_out[128:216, :])

            for j in range(HB):
                hook = cc1_hook if (j == HB - 1 and HB > 1) else None
                prods = emit_products(j, mid_hook=hook)
                emit_reduce_row(j, prods)

            # ---- last row partials -> CC2 (small, warm, aligned) ----
            nc.vector.tensor_reduce(pA2[:], accA[:, early:n_drain],
                                    mybir.AxisListType.X, ALU.add)
            nc.vector.tensor_reduce(pB2[:], accB[:, early:n_drain],
                                    mybir.AxisListType.X, ALU.add)
            nc.gpsimd.dma_start(cc2_in[0:128, :], pA2[:])
            nc.gpsimd.dma_start(cc2_in[128:216, :], pB2[:])
            nc.gpsimd.collective_compute(
                "AllReduce", ALU.add,
                replica_groups=[list(range(N_CORES))],
                ins=[cc2_in[:].opt()],
                outs=[cc2_out[:].opt()],
            )
            nc.gpsimd.dma_start(pAg[:, 1:2], cc2_out[0:128, :])
            nc.gpsimd.dma_start(pBg[:, 1:2], cc2_out[128:216, :])
            if HB == 1:
                nc.vector.tensor_scalar(pAg[:, 0:1], pA2[:], 0.0, None,
                                        ALU.mult)
                nc.vector.tensor_scalar(pBg[:, 0:1], pB2[:], 0.0, None,
                                        ALU.mult)

            # ---- gate MLP ----
            hps = ps.tile([MID, 1], FP32, tag="psA0", padded_shape=[128, 512])
            nc.tensor.matmul(hps[:], w0at[:], pAg[:, 0:1],
                             start=True, stop=False)
            nc.tensor.matmul(hps[:], w0bt[:], pBg[:, 0:1],
                             start=False, stop=False)
            nc.tensor.matmul(hps[:], w0at[:], pAg[:, 1:2],
                             start=False, stop=False)
            nc.tensor.matmul(hps[:], w0bt[:], pBg[:, 1:2],
                             start=False, stop=True)
            h0 = cpool.tile([MID, 1], FP32)
            hvec = cpool.tile([MID, 1], FP32)
            nc.vector.tensor_tensor(h0[:], hps[:], b0t[:], ALU.add)
            nc.vector.tensor_scalar(hvec[:], h0[:], 0.0, None, ALU.max)
            gpsA = ps.tile([128, 1], FP32, tag="psA1", padded_shape=[128, 512])
            gpsB = ps.tile([88, 1], FP32, tag="psA2", padded_shape=[128, 512])
            nc.tensor.matmul(gpsA[:], w1at[:], hvec[:], start=True, stop=True)
            nc.tensor.matmul(gpsB[:], w1bt[:], hvec[:], start=True, stop=True)
            gA = cpool.tile([128, 1], FP32)
            gB = cpool.tile([88, 1], FP32)
            nc.scalar.activation(gA[:], gpsA[:], AF.Sigmoid, bias=b1at[:],
                                 scale=1.0)
            nc.scalar.activation(gB[:], gpsB[:], AF.Sigmoid, bias=b1bt[:],
                                 scale=1.0)

            # ---- gated writeout. Gating on ACT (1/3 of A rows) + DVE;
            # full-row DMAs on HWDGE queues only (sync + scalar). ----
            for j in range(HB):
                stA = spool.tile([128, FD], BF16, tag="gsA", bufs=4)
                if j % 3 == 0:
                    nc.scalar.mul(stA[:], corrA[:, j, :], gA[:])
                else:
                    nc.vector.tensor_scalar(stA[:], corrA[:, j, :], gA[:],
                                            None, ALU.mult)
                stB = spool.tile([88, FD], BF16, tag="gsB", bufs=4)
                nc.vector.tensor_scalar(stB[:], corrB[:, j, :], gB[:],
                                        None, ALU.mult)
                nc.sync.dma_start(out_d[j, 0:128, :], stA[:])
                (nc.sync if j % 2 == 0 else nc.scalar).dma_start(
                    out_d[j, 128:216, :], stB[:])

    nc.compile()
    return nc


# ---------------- host-side prep / assembly ----------------

def make_gate_consts(w0, b0, w1, b1, cfg: Cfg):
    norm = 1.0 / (cfg.W * cfg.H * cfg.D)
    sel = np.zeros((128, 128), dtype=np.float32)
    for v in range(4):
        for c in range(C):
            for h8 in range(H8):
                sel[c * H8 + h8, 32 * v + 8 * v + h8] = 1.0 / 16
    w0 = np.asarray(w0, dtype=np.float32)
    w1 = np.asarray(w1, dtype=np.float32)
    b1 = np.asarray(b1, dtype=np.float32)
    w0a = np.zeros((128, MID), dtype=np.float32)
    w0b = np.zeros((88, MID), dtype=np.float32)
    w1ra = np.zeros((MID, 128), dtype=np.float32)
    w1rb = np.zeros((MID, 88), dtype=np.float32)
    b1ra = np.zeros((128, 1), dtype=np.float32)
    b1rb = np.zeros((88, 1), dtype=np.float32)
    for k in range(K):
        for h8 in range(H8):
            r = _row_of(k, h8)
            if k < 16:
                w0a[r, :] = w0[:, k] * norm
                w1ra[:, r] = w1[k, :]
                b1ra[r, 0] = b1[k]
            else:
                w0b[r - 128, :] = w0[:, k] * norm
                w1rb[:, r - 128] = w1[k, :]
                b1rb[r - 128, 0] = b1[k]
    return {
        "selmats": sel.astype(ml_dtypes.bfloat16),
        "w0a": w0a, "w0b": w0b, "w1ra": w1ra, "w1rb": w1rb,
        "b0c": np.asarray(b0, dtype=np.float32).reshape(MID, 1),
        "b1ra": b1ra, "b1rb": b1rb,
    }


def _fold(a, HB):
    # [C, w, H, D'] -> [(c h8), hblk, w, d]
    Cc, ww, hh, dd = a.shape
    a = a.reshape(Cc, ww, H8, HB, dd)
    a = np.ascontiguousarray(a.transpose(0, 2, 3, 1, 4))
    return a.reshape(C * H8, HB, ww, dd)


def make_inputs_per_core(x_1, x_2, w0, b0, w1, b1, cfg: Cfg):
    """x_1/x_2: [1, C, W, H, D] float32 -> list of per-core input dicts."""
    W, H, D, De = cfg.W, cfg.H, cfg.D, cfg.De
    Wc, HB = cfg.Wc, cfg.HB
    x1 = np.asarray(x_1)[0].astype(ml_dtypes.bfloat16)      # [C, W, H, D]
    x2 = np.asarray(x_2)[0].astype(ml_dtypes.bfloat16)
    # padded x2: w +-1, h +-1, d in [-1, D+1)
    x2p = np.zeros((C, W + 2, H + 2, D + 2), dtype=ml_dtypes.bfloat16)
    x2p[:, 1:W + 1, 1:H + 1, 1:D + 1] = x2
    # hblk-extended h indices: row r of (h8) block = x2p h-index h8*HB + r,
    # covering h = h8*HB - 1 .. (h8+1)*HB (1-voxel halo on both sides)
    hidx = (np.arange(H8) * HB)[:, None] + np.arange(HB + 2)  # [H8, HB+2]

    consts = make_gate_consts(w0, b0, w1, b1, cfg)
    in_maps = []
    for ci in range(N_CORES):
        ws = ci * Wc
        m = dict(consts)
        m["x1"] = np.ascontiguousarray(
            _fold(x1[:, ws:ws + Wc, :, :], HB).transpose(1, 0, 2, 3))
        blk = x2p[:, ws:ws + Wc + 2, :, :]                  # [C, Wc+2, H+2, De]
        oo = blk[:, :, hidx, 0:De]                          # [C, Wc+2, H8, HB+2, De]
        m["x2"] = np.ascontiguousarray(
            oo.transpose(3, 0, 2, 1, 4)).reshape(HB + 2, 128, Wc + 2, De)
        in_maps.append(m)
    return in_maps


def assemble_output(results, cfg: Cfg):
    W, H, D = cfg.W, cfg.H, cfg.D
    Wc, HB = cfg.Wc, cfg.HB
    rows = np.empty((K, H8), dtype=np.int64)
    for k in range(K):
        for h8 in range(H8):
            rows[k, h8] = _row_of(k, h8)
    out = np.empty((W, H, D, K), dtype=np.float32)
    for ci, r in enumerate(results):
        o = np.asarray(r["out"]).reshape(HB, 216, Wc, D)
        o = o.transpose(1, 0, 2, 3)
        core = o[rows]                        # [K, H8, HB, Wc, D]
        core = core.transpose(3, 1, 2, 4, 0)  # [Wc, H8, HB, D, K]
        out[ci * Wc:(ci + 1) * Wc] = core.reshape(Wc, H, D, K)
    return out[None]


_CACHE = {}
TRACE = False           # test harness can set kernel.TRACE = True


def kernel(x_1, x_2, w0, b0, w1, b1):
    cfg = Cfg()
    if "nc" not in _CACHE:
        _CACHE["nc"] = build_nc(cfg)
    nc = _CACHE["nc"]
    in_maps = make_inputs_per_core(x_1, x_2, w0, b0, w1, b1, cfg)
    last_exc = None
    for _attempt in range(3):
        try:
            res = run_bass_kernel_spmd(nc, in_maps,
                                       core_ids=list(range(N_CORES)),
                                       trace=TRACE)
            break
        except Exception as e:  # transient NRT device errors: retry
            last_exc = e
    else:
        raise last_exc
    _CACHE["last_res"] = res
    return assemble_output(res.results, cfg)


# revision 8
# speedup vs baseline: 1.1585x; 1.0395x over previous
"""Trainium2 Bass kernel for shifted-window correlation (27 shifts) + SE gate.

Reference computation (shapes hardcoded; B=1, C=16, W=80, H=96, D=112):
  corr[w,h,d,k] = mean_c x1[c,w,h,d] * x2[c, w+sx, h+sy, d+sz]   (zero-padded)
  s = mean_{w,h,d} corr;  g = sigmoid(w1 @ relu(w0 @ s + b0) + b1)
  out = corr * g

Strategy (8 cores, W sharded 10/core), v2:
  - SBUF partition dim = (c:16, h8:8) where h8 = h // (H/8).
  - ONE resident x2 tile per core ([128, HB+2, Wc+2, De], De=D+2) whose
    hblk axis carries a 1-row halo; all 27 shifts are free-dim offsets.
    (bf16 DVE speed is offset-alignment-independent — measured — so no
    even/odd d copies.)
  - Products on DVE (bf16 2x mode, ~0.52ns/elem): one fused op per
    (sx,sy) computing all three dz via an overlapping unit-stride AP
    (3 free dims is the TensorTensor ISA limit) -> 9 ops/row.
  - Channel reduction on PE via a block-diagonal selection matmul
    packing (k,h8) into 128/88-row PSUM tiles; natural k order so
    consecutive matmuls cycle the 3 slice banks; A banks drain on ACT
    (capturing squeeze partials via accum_out) right after k=15 so the
    next row never stalls on PSUM.
  - Squeeze allreduce split in two: CC1 ([216,1], rows 0..HB-2) fires
    mid-way through the last row's products and absorbs cross-core
    skew; CC2 ([216,1], last row) right after the final drain only
    pays warm-latency (~9us). Gate MLP accumulates both columns.
  - relu on DVE + sigmoid table preloaded at ramp so the tail has no
    ACT_TABLE_LOAD.
  - Gated writeout overlaps: ACT gates 1/3 of A rows + DVE the rest;
    full-row output DMAs on HWDGE queues only (sync + a few on scalar)
    -- gpsimd SWDGE would starve against DVE perf-mode gating ops.
"""

import sys
import types

import numpy as np
import ml_dtypes


def _install_ntff_hook_shim():
    """agent image's antenv lacks axon_hooks; needed only for trace=True."""
    if "antenv.axon_hooks" in sys.modules:
        return
    try:
        import antenv
        from trn_agent_boot.trn_boot import _ntff_profile_via_ctypes

        hook = _ntff_profile_via_ctypes("/opt/axon/libaxon_pjrt.so")
        mod = types.ModuleType("antenv.axon_hooks")
        ref = {"h": hook}
        mod.get_axon_ntff_profile_hook = lambda: ref["h"]
        mod.set_axon_ntff_profile_hook = lambda h: ref.__setitem__("h", h)
        sys.modules["antenv.axon_hooks"] = mod
        antenv.axon_hooks = mod
    except Exception:
        pass


_install_ntff_hook_shim()

import concourse.bacc as bacc  # noqa: E402
import concourse.tile as tile  # noqa: E402
import concourse.mybir as mybir  # noqa: E402
from concourse.ap import AP as _AP  # noqa: E402
from concourse.bass_utils import run_bass_kernel_spmd  # noqa: E402

BF16 = mybir.dt.bfloat16
FP32 = mybir.dt.float32
AF = mybir.ActivationFunctionType
ALU = mybir.AluOpType

N_CORES = 8
C = 16
H8 = 8          # partition sub-dim over h
K = 27
MID = 6


class Cfg:
    def __init__(self, W=80, H=96, D=112):
        assert H % H8 == 0
        self.W, self.H, self.D = W, H, D
        self.Wc = W // N_CORES          # w columns per core
        self.HB = H // H8               # hblk extent (free dim)
        self.De = D + 2                 # padded d extent
        self.FD = self.Wc * D           # flat (w, d) free size per row
        self.slices = [(o, min(o + 512, self.FD))
                       for o in range(0, self.FD, 512)]
        self.n_fs = len(self.slices)


# shift order matches reference: k = dx*9 + dy*3 + dz, s* = d*-1
SHIFTS = [(dx - 1, dy - 1, dz - 1)
          for dx in range(3) for dy in range(3) for dz in range(3)]


def _gv_of(k):
    """(is_A, psum column group, selection slice) for shift k."""
    kk = k if k < 16 else k - 16
    return k < 16, kk // 4, kk % 4


def _row_of(k, h8):
    """corr partition row for (k, h8). Tile A: k 0..15, tile B: 16..26."""
    kk = k if k < 16 else k - 16
    base = 0 if k < 16 else 128
    return base + 32 * (kk // 4) + 8 * (kk % 4) + h8


def build_nc(cfg: Cfg):
    nc = bacc.Bacc("TRN2", target_bir_lowering=False, debug=False,
                   num_devices=N_CORES)
    HB, Wc, D, De, FD = cfg.HB, cfg.Wc, cfg.D, cfg.De, cfg.FD

    x1_d = nc.dram_tensor("x1", [HB, 128, Wc, D], BF16, kind="ExternalInput")
    x2_d = nc.dram_tensor("x2", [HB + 2, 128, Wc + 2, De], BF16,
                          kind="ExternalInput")
    sel_d = nc.dram_tensor("selmats", [128, 128], BF16, kind="ExternalInput")
    w0a_d = nc.dram_tensor("w0a", [128, MID], FP32, kind="ExternalInput")
    w0b_d = nc.dram_tensor("w0b", [88, MID], FP32, kind="ExternalInput")
    w1a_d = nc.dram_tensor("w1ra", [MID, 128], FP32, kind="ExternalInput")
    w1b_d = nc.dram_tensor("w1rb", [MID, 88], FP32, kind="ExternalInput")
    b0_d = nc.dram_tensor("b0c", [MID, 1], FP32, kind="ExternalInput")
    b1a_d = nc.dram_tensor("b1ra", [128, 1], FP32, kind="ExternalInput")
    b1b_d = nc.dram_tensor("b1rb", [88, 1], FP32, kind="ExternalInput")
    out_d = nc.dram_tensor("out", [HB, 216, FD], BF16, kind="ExternalOutput")

    n_drain = HB * cfg.n_fs
    # CC1 fires mid-way through row `jh`'s products and covers rows 0..jh-1;
    # leaving ~3 rows (~50us) after it absorbs cross-core jitter + CC latency
    # well before CC2's input is ready.
    jh = max(1, HB - 4)
    early = jh * cfg.n_fs           # accum cols covered by CC1

    with tile.TileContext(nc) as tc:
        with (
            tc.tile_pool(name="const", bufs=1) as cpool,
            tc.tile_pool(name="ps", bufs=1, space="PSUM") as ps,
            tc.tile_pool(name="dram", bufs=1, space="DRAM") as dram,
            tc.tile_pool(name="pp", bufs=6) as ppool,
            tc.tile_pool(name="stage", bufs=4) as spool,
        ):
            # resident tiles
            x1t = cpool.tile([128, HB, Wc, D], BF16)
            x2t = cpool.tile([128, HB + 2, Wc + 2, De], BF16)
            corrA = cpool.tile([128, HB, FD], BF16)
            corrB = cpool.tile([88, HB, FD], BF16)
            selt = cpool.tile([128, 128], BF16)
            w0at = cpool.tile([128, MID], FP32)
            w0bt = cpool.tile([88, MID], FP32)
            w1at = cpool.tile([MID, 128], FP32)
            w1bt = cpool.tile([MID, 88], FP32)
            b0t = cpool.tile([MID, 1], FP32)
            b1at = cpool.tile([128, 1], FP32)
            b1bt = cpool.tile([88, 1], FP32)
            accA = cpool.tile([128, n_drain], FP32)
            accB = cpool.tile([88, n_drain], FP32)
            pA1 = cpool.tile([128, 1], FP32)
            pB1 = cpool.tile([88, 1], FP32)
            pA2 = cpool.tile([128, 1], FP32)
            pB2 = cpool.tile([88, 1], FP32)
            pAg = cpool.tile([128, 2], FP32)
            pBg = cpool.tile([88, 2], FP32)
            dumin = cpool.tile([1, 2], FP32)
            dumout = cpool.tile([1, 2], FP32)

            rr_state = [0]
            rr_engs = [nc.gpsimd, nc.sync, nc.scalar]

            def _rr():
                e = rr_engs[rr_state[0] % len(rr_engs)]
                rr_state[0] += 1
                return e

            def load_x1_row(r, parts=1, eng=None):
                cuts = [round(i * Wc / parts) for i in range(parts + 1)]
                for a, b in zip(cuts, cuts[1:]):
                    if a < b:
                        (eng or _rr()).dma_start(x1t[:, r, a:b, :],
                                                 x1_d[r, :, a:b, :])

            def load_x2_row(r, parts=1, eng=None):
                cuts = [round(i * (Wc + 2) / parts) for i in range(parts + 1)]
                for a, b in zip(cuts, cuts[1:]):
                    if a < b:
                        (eng or _rr()).dma_start(x2t[:, r, a:b, :],
                                                 x2_d[r, :, a:b, :])

            # priority ramp: first product (dx=-1,dy=-1) needs x2 row 0 +
            # x1 row 0. Two big halves each on the sync+gpsimd queues;
            # scalar preloads the sigmoid ACT table (tail then pays no
            # table swap) and loads selt, then joins the DMA round-robin.
            wh = (Wc + 2) // 2
            xh = max(Wc // 2, 1)
            nc.sync.dma_start(x2t[:, 0, 0:wh, :], x2_d[0, :, 0:wh, :])
            nc.gpsimd.dma_start(x2t[:, 0, wh:Wc + 2, :],
                                x2_d[0, :, wh:Wc + 2, :])
            nc.sync.dma_start(x1t[:, 0, 0:xh, :], x1_d[0, :, 0:xh, :])
            nc.gpsimd.dma_start(x1t[:, 0, xh:Wc, :], x1_d[0, :, xh:Wc, :])
            nc.gpsimd.memset(dumin[:], 0.0)
            nc.scalar.activation(dumout[:], dumin[:], AF.Sigmoid)
            nc.scalar.dma_start(selt[:], sel_d[:])
            rr_state[0] = 0
            load_x2_row(1, parts=3)
            load_x2_row(2, parts=3)
            if HB > 1:
                load_x1_row(1, parts=3)
            load_x2_row(3, parts=2)
            nc.sync.dma_start(w0at[:], w0a_d[:])
            nc.sync.dma_start(w0bt[:], w0b_d[:])
            nc.sync.dma_start(w1at[:], w1a_d[:])
            nc.sync.dma_start(w1bt[:], w1b_d[:])
            nc.sync.dma_start(b0t[:], b0_d[:])
            nc.sync.dma_start(b1at[:], b1a_d[:])
            nc.sync.dma_start(b1bt[:], b1b_d[:])

            # Warm-up collective: absorbs cross-core launch skew and CC
            # firmware setup so the real allreduces only pay marginal cost.
            warm_in = dram.tile([MID, 1], FP32)
            warm_out = dram.tile([MID, 1], FP32)
            nc.sync.dma_start(warm_in[:], b0_d[:])
            nc.gpsimd.collective_compute(
                "AllReduce", ALU.add,
                replica_groups=[list(range(N_CORES))],
                ins=[warm_in[:].opt()],
                outs=[warm_out[:].opt()],
            )

            # remaining loads, ~one row ahead of first use
            nx1 = 2
            for r in range(4, HB + 2):
                if nx1 < min(r - 1, HB):
                    load_x1_row(nx1)
                    nx1 += 1
                load_x2_row(r)
            while nx1 < HB:
                load_x1_row(nx1)
                nx1 += 1

            cc1_in = dram.tile([216, 1], FP32)
            cc1_out = dram.tile([216, 1], FP32)
            cc2_in = dram.tile([216, 1], FP32)
            cc2_out = dram.tile([216, 1], FP32)

            a_tot = {g: 4 for g in range(4)}
            b_tot = {0: 4, 1: 4, 2: 3}

            def emit_products(j, mid_hook=None):
                """9 fused (sx,sy) product ops for row j; each computes all
                three dz shifts via an overlapping unit-stride AP."""
                prods = {}
                for t, (dx, dy) in enumerate(
                        (a, b) for a in range(3) for b in range(3)):
                    if t == 4 and mid_hook is not None:
                        mid_hook()
                    pt = ppool.tile([128, 3, FD], BF16, tag="P", bufs=6)
                    base = x2t[:, j + dy, dx:dx + Wc, 0:De]
                    u = base.unsqueeze(1)
                    ap2 = [list(p) for p in u.ap]
                    ap2[1] = [1, 3]
                    ap2[3] = [1, D]
                    srcv = _AP(tensor=u.tensor, offset=u.offset, ap=ap2)
                    x1b = x1t[:, j].unsqueeze(1).broadcast_to([128, 3, Wc, D])
                    dst = pt[:].rearrange("p r (w d) -> p r w d", d=D)
                    nc.vector.tensor_tensor(dst, x1b, srcv, ALU.mult)
                    kb = 9 * dx + 3 * dy
                    for dz in range(3):
                        prods[kb + dz] = (pt, dz)
                return prods

            def emit_reduce_row(j, prods):
                """PE reduction + ACT drains for row j, natural k order."""
                psA = [ps.tile([128, hi - lo], FP32, tag=f"psA{i}",
                               name=f"psA{i}", padded_shape=[128, 512])
                       for i, (lo, hi) in enumerate(cfg.slices)]
                psB = [ps.tile([128, hi - lo], FP32, tag=f"psB{i}",
                               name=f"psB{i}", padded_shape=[128, 512])
                       for i, (lo, hi) in enumerate(cfg.slices)]
                seen = {}
                for k in range(K):
                    is_a, g, v = _gv_of(k)
                    tot = a_tot[g] if is_a else b_tot[g]
                    cnt = seen.get((is_a, g), 0)
                    seen[(is_a, g)] = cnt + 1
                    pst = psA if is_a else psB
                    pk, idx = prods[k]
                    for i, (lo, hi) in enumerate(cfg.slices):
                        nc.tensor.matmul(
                            pst[i][32 * g:32 * g + 32, :],
                            selt[:, 32 * v:32 * v + 32],
                            pk[:, idx, lo:hi],
                            start=(cnt == 0), stop=(cnt == tot - 1),
                            tile_position=(0, 32 * g),
                        )
                    if k == 15:
                        # A chains complete: drain now so the next row's
                        # A matmuls never wait on these banks.
                        for i, (lo, hi) in enumerate(cfg.slices):
                            di = j * cfg.n_fs + i
                            nc.scalar.activation(
                                corrA[:, j, lo:hi], psA[i][:], AF.Copy,
                                accum_out=accA[:, di:di + 1])
                for i, (lo, hi) in enumerate(cfg.slices):
                    di = j * cfg.n_fs + i
                    nc.scalar.activation(
                        corrB[:, j, lo:hi], psB[i][0:88, :], AF.Copy,
                        accum_out=accB[:, di:di + 1])

            def cc1_hook():
                # rows 0..HB-2 partial sums: reduce on DVE, allreduce while
                # the last row's products still run (absorbs cross-core skew)
                nc.vector.tensor_reduce(pA1[:], accA[:, 0:early],
                                        mybir.AxisListType.X, ALU.add)
                nc.vector.tensor_reduce(pB1[:], accB[:, 0:early],
                                        mybir.AxisListType.X, ALU.add)
                nc.gpsimd.dma_start(cc1_in[0:128, :], pA1[:])
                nc.gpsimd.dma_start(cc1_in[128:216, :], pB1[:])
                nc.gpsimd.collective_compute(
                    "AllReduce", ALU.add,
                    replica_groups=[list(range(N_CORES))],
                    ins=[cc1_in[:].opt()],
                    outs=[cc1_out[:].opt()],
                )
                nc.gpsimd.dma_start(pAg[:, 0:1], cc1_out[0:128, :])
                nc.gpsimd.dma_start(pBg[:, 0:1], cc1_out[128:216, :])

            for j in range(HB):
                hook = cc1_hook if (j == jh and HB > 1) else None
                prods = emit_products(j, mid_hook=hook)
                emit_reduce_row(j, prods)

            # ---- last row partials -> CC2 (small, warm, aligned) ----
            nc.vector.tensor_reduce(pA2[:], accA[:, early:n_drain],
                                    mybir.AxisListType.X, ALU.add)
            nc.vector.tensor_reduce(pB2[:], accB[:, early:n_drain],
                                    mybir.AxisListType.X, ALU.add)
            nc.gpsimd.dma_start(cc2_in[0:128, :], pA2[:])
            nc.gpsimd.dma_start(cc2_in[128:216, :], pB2[:])
            nc.gpsimd.collective_compute(
                "AllReduce", ALU.add,
                replica_groups=[list(range(N_CORES))],
                ins=[cc2_in[:].opt()],
                outs=[cc2_out[:].opt()],
            )
            nc.sync.dma_start(pAg[:, 1:2], cc2_out[0:128, :])
            nc.sync.dma_start(pBg[:, 1:2], cc2_out[128:216, :])
            if HB == 1:
                nc.vector.tensor_scalar(pAg[:, 0:1], pA2[:], 0.0, None,
                                        ALU.mult)
                nc.vector.tensor_scalar(pBg[:, 0:1], pB2[:], 0.0, None,
                                        ALU.mult)

            # ---- gate MLP ----
            hps = ps.tile([MID, 1], FP32, tag="psA0", padded_shape=[128, 512])
            nc.tensor.matmul(hps[:], w0at[:], pAg[:, 0:1],
                             start=True, stop=False)
            nc.tensor.matmul(hps[:], w0bt[:], pBg[:, 0:1],
                             start=False, stop=False)
            nc.tensor.matmul(hps[:], w0at[:], pAg[:, 1:2],
                             start=False, stop=False)
            nc.tensor.matmul(hps[:], w0bt[:], pBg[:, 1:2],
                             start=False, stop=True)
            h0 = cpool.tile([MID, 1], FP32)
            hvec = cpool.tile([MID, 1], FP32)
            nc.vector.tensor_tensor(h0[:], hps[:], b0t[:], ALU.add)
            nc.vector.tensor_scalar(hvec[:], h0[:], 0.0, None, ALU.max)
            gpsA = ps.tile([128, 1], FP32, tag="psA1", padded_shape=[128, 512])
            gpsB = ps.tile([88, 1], FP32, tag="psA2", padded_shape=[128, 512])
            nc.tensor.matmul(gpsA[:], w1at[:], hvec[:], start=True, stop=True)
            nc.tensor.matmul(gpsB[:], w1bt[:], hvec[:], start=True, stop=True)
            gA = cpool.tile([128, 1], FP32)
            gB = cpool.tile([88, 1], FP32)
            nc.scalar.activation(gA[:], gpsA[:], AF.Sigmoid, bias=b1at[:],
                                 scale=1.0)
            nc.scalar.activation(gB[:], gpsB[:], AF.Sigmoid, bias=b1bt[:],
                                 scale=1.0)

            # ---- gated writeout. Gating on ACT (1/3 of A rows) + DVE;
            # full-row DMAs on HWDGE queues only (sync + scalar). ----
            for j in range(HB):
                stA = spool.tile([128, FD], BF16, tag="gsA", bufs=6)
                if j % 3 == 0:
                    nc.scalar.mul(stA[:], corrA[:, j, :], gA[:])
                else:
                    nc.vector.tensor_scalar(stA[:], corrA[:, j, :], gA[:],
                                            None, ALU.mult)
                stB = spool.tile([88, FD], BF16, tag="gsB", bufs=6)
                nc.vector.tensor_scalar(stB[:], corrB[:, j, :], gB[:],
                                        None, ALU.mult)
                nc.sync.dma_start(out_d[j, 0:128, :], stA[:])
                (nc.sync if j % 2 == 0 else nc.scalar).dma_start(
                    out_d[j, 128:216, :], stB[:])

    nc.compile()
    return nc


# ---------------- host-side prep / assembly ----------------

def make_gate_consts(w0, b0, w1, b1, cfg: Cfg):
    norm = 1.0 / (cfg.W * cfg.H * cfg.D)
    sel = np.zeros((128, 128), dtype=np.float32)
    for v in range(4):
        for c in range(C):
            for h8 in range(H8):
                sel[c * H8 + h8, 32 * v + 8 * v + h8] = 1.0 / 16
    w0 = np.asarray(w0, dtype=np.float32)
    w1 = np.asarray(w1, dtype=np.float32)
    b1 = np.asarray(b1, dtype=np.float32)
    w0a = np.zeros((128, MID), dtype=np.float32)
    w0b = np.zeros((88, MID), dtype=np.float32)
    w1ra = np.zeros((MID, 128), dtype=np.float32)
    w1rb = np.zeros((MID, 88), dtype=np.float32)
    b1ra = np.zeros((128, 1), dtype=np.float32)
    b1rb = np.zeros((88, 1), dtype=np.float32)
    for k in range(K):
        for h8 in range(H8):
            r = _row_of(k, h8)
            if k < 16:
                w0a[r, :] = w0[:, k] * norm
                w1ra[:, r] = w1[k, :]
                b1ra[r, 0] = b1[k]
            else:
                w0b[r - 128, :] = w0[:, k] * norm
                w1rb[:, r - 128] = w1[k, :]
                b1rb[r - 128, 0] = b1[k]
    return {
        "selmats": sel.astype(ml_dtypes.bfloat16),
        "w0a": w0a, "w0b": w0b, "w1ra": w1ra, "w1rb": w1rb,
        "b0c": np.asarray(b0, dtype=np.float32).reshape(MID, 1),
        "b1ra": b1ra, "b1rb": b1rb,
    }


def _fold(a, HB):
    # [C, w, H, D'] -> [(c h8), hblk, w, d]
    Cc, ww, hh, dd = a.shape
    a = a.reshape(Cc, ww, H8, HB, dd)
    a = np.ascontiguousarray(a.transpose(0, 2, 3, 1, 4))
    return a.reshape(C * H8, HB, ww, dd)


def make_inputs_per_core(x_1, x_2, w0, b0, w1, b1, cfg: Cfg):
    """x_1/x_2: [1, C, W, H, D] float32 -> list of per-core input dicts."""
    W, H, D, De = cfg.W, cfg.H, cfg.D, cfg.De
    Wc, HB = cfg.Wc, cfg.HB
    x1 = np.asarray(x_1)[0].astype(ml_dtypes.bfloat16)      # [C, W, H, D]
    x2 = np.asarray(x_2)[0].astype(ml_dtypes.bfloat16)
    # padded x2: w +-1, h +-1, d in [-1, D+1)
    x2p = np.zeros((C, W + 2, H + 2, D + 2), dtype=ml_dtypes.bfloat16)
    x2p[:, 1:W + 1, 1:H + 1, 1:D + 1] = x2
    # hblk-extended h indices: row r of (h8) block = x2p h-index h8*HB + r,
    # covering h = h8*HB - 1 .. (h8+1)*HB (1-voxel halo on both sides)
    hidx = (np.arange(H8) * HB)[:, None] + np.arange(HB + 2)  # [H8, HB+2]

    consts = make_gate_consts(w0, b0, w1, b1, cfg)
    in_maps = []
    for ci in range(N_CORES):
        ws = ci * Wc
        m = dict(consts)
        m["x1"] = np.ascontiguousarray(
            _fold(x1[:, ws:ws + Wc, :, :], HB).transpose(1, 0, 2, 3))
        blk = x2p[:, ws:ws + Wc + 2, :, :]                  # [C, Wc+2, H+2, De]
        oo = blk[:, :, hidx, 0:De]                          # [C, Wc+2, H8, HB+2, De]
        m["x2"] = np.ascontiguousarray(
            oo.transpose(3, 0, 2, 1, 4)).reshape(HB + 2, 128, Wc + 2, De)
        in_maps.append(m)
    return in_maps


def assemble_output(results, cfg: Cfg):
    W, H, D = cfg.W, cfg.H, cfg.D
    Wc, HB = cfg.Wc, cfg.HB
    rows = np.empty((K, H8), dtype=np.int64)
    for k in range(K):
        for h8 in range(H8):
            rows[k, h8] = _row_of(k, h8)
    out = np.empty((W, H, D, K), dtype=np.float32)
    for ci, r in enumerate(results):
        o = np.asarray(r["out"]).reshape(HB, 216, Wc, D)
        o = o.transpose(1, 0, 2, 3)
        core = o[rows]                        # [K, H8, HB, Wc, D]
        core = core.transpose(3, 1, 2, 4, 0)  # [Wc, H8, HB, D, K]
        out[ci * Wc:(ci + 1) * Wc] = core.reshape(Wc, H, D, K)
    return out[None]


_CACHE = {}
TRACE = False           # test harness can set kernel.TRACE = True


def kernel(x_1, x_2, w0, b0, w1, b1):
    cfg = Cfg()
    if "nc" not in _CACHE:
        _CACHE["nc"] = build_nc(cfg)
    nc = _CACHE["nc"]
    in_maps = make_inputs_per_core(x_1, x_2, w0, b0, w1, b1, cfg)
    last_exc = None
    for _attempt in range(3):
        try:
            res = run_bass_kernel_spmd(nc, in_maps,
                                       core_ids=list(range(N_CORES)),
                                       trace=TRACE)
            break
        except Exception as e:  # transient NRT device errors: retry
            last_exc = e
    else:
        raise last_exc
    _CACHE["last_res"] = res
    return assemble_output(res.results, cfg)


# revision 21
# speedup vs baseline: 1.1771x; 1.0161x over previous
"""Trainium2 Bass kernel for shifted-window correlation (27 shifts) + SE gate.

Reference computation (shapes hardcoded; B=1, C=16, W=80, H=96, D=112):
  corr[w,h,d,k] = mean_c x1[c,w,h,d] * x2[c, w+sx, h+sy, d+sz]   (zero-padded)
  s = mean_{w,h,d} corr;  g = sigmoid(w1 @ relu(w0 @ s + b0) + b1)
  out = corr * g

Strategy (8 cores, W sharded 10/core), v2:
  - SBUF partition dim = (c:16, h8:8) where h8 = h // (H/8).
  - ONE resident x2 tile per core ([128, HB+2, Wc+2, De], De=D+2) whose
    hblk axis carries a 1-row halo; all 27 shifts are free-dim offsets.
    (bf16 DVE speed is offset-alignment-independent — measured — so no
    even/odd d copies.)
  - Products on DVE (bf16 2x mode, ~0.52ns/elem): one fused op per
    (sx,sy) computing all three dz via an overlapping unit-stride AP
    (3 free dims is the TensorTensor ISA limit) -> 9 ops/row.
  - Channel reduction on PE via a block-diagonal selection matmul
    packing (k,h8) into 128/88-row PSUM tiles; natural k order so
    consecutive matmuls cycle the 3 slice banks; A banks drain on ACT
    (capturing squeeze partials via accum_out) right after k=15 so the
    next row never stalls on PSUM.
  - Squeeze allreduce split in two: CC1 ([216,1], rows 0..HB-2) fires
    mid-way through the last row's products and absorbs cross-core
    skew; CC2 ([216,1], last row) right after the final drain only
    pays warm-latency (~9us). Gate MLP accumulates both columns.
  - relu on DVE + sigmoid table preloaded at ramp so the tail has no
    ACT_TABLE_LOAD.
  - Gated writeout overlaps: ACT gates 1/3 of A rows + DVE the rest;
    full-row output DMAs on HWDGE queues only (sync + a few on scalar)
    -- gpsimd SWDGE would starve against DVE perf-mode gating ops.
"""

import sys
import types

import numpy as np
import ml_dtypes


def _install_ntff_hook_shim():
    """agent image's antenv lacks axon_hooks; needed only for trace=True."""
    if "antenv.axon_hooks" in sys.modules:
        return
    try:
        import antenv
        from trn_agent_boot.trn_boot import _ntff_profile_via_ctypes

        hook = _ntff_profile_via_ctypes("/opt/axon/libaxon_pjrt.so")
        mod = types.ModuleType("antenv.axon_hooks")
        ref = {"h": hook}
        mod.get_axon_ntff_profile_hook = lambda: ref["h"]
        mod.set_axon_ntff_profile_hook = lambda h: ref.__setitem__("h", h)
        sys.modules["antenv.axon_hooks"] = mod
        antenv.axon_hooks = mod
    except Exception:
        pass


_install_ntff_hook_shim()

import concourse.bacc as bacc  # noqa: E402
import concourse.tile as tile  # noqa: E402
import concourse.mybir as mybir  # noqa: E402
from concourse.ap import AP as _AP  # noqa: E402
from concourse.bass_utils import run_bass_kernel_spmd  # noqa: E402

BF16 = mybir.dt.bfloat16
FP32 = mybir.dt.float32
AF = mybir.ActivationFunctionType
ALU = mybir.AluOpType

N_CORES = 8
C = 16
H8 = 8          # partition sub-dim over h
K = 27
MID = 6


class Cfg:
    def __init__(self, W=80, H=96, D=112):
        assert H % H8 == 0
        self.W, self.H, self.D = W, H, D
        self.Wc = W // N_CORES          # w columns per core
        self.HB = H // H8               # hblk extent (free dim)
        self.De = D + 2                 # padded d extent
        self.FD = self.Wc * D           # flat (w, d) free size per row
        self.slices = [(o, min(o + 512, self.FD))
                       for o in range(0, self.FD, 512)]
        self.n_fs = len(self.slices)


# shift order matches reference: k = dx*9 + dy*3 + dz, s* = d*-1
SHIFTS = [(dx - 1, dy - 1, dz - 1)
          for dx in range(3) for dy in range(3) for dz in range(3)]


def _gv_of(k):
    """(is_A, psum column group, selection slice) for shift k."""
    kk = k if k < 16 else k - 16
    return k < 16, kk // 4, kk % 4


def _row_of(k, h8):
    """corr partition row for (k, h8). Tile A: k 0..15, tile B: 16..26."""
    kk = k if k < 16 else k - 16
    base = 0 if k < 16 else 128
    return base + 32 * (kk // 4) + 8 * (kk % 4) + h8


def build_nc(cfg: Cfg):
    nc = bacc.Bacc("TRN2", target_bir_lowering=False, debug=False,
                   num_devices=N_CORES)
    HB, Wc, D, De, FD = cfg.HB, cfg.Wc, cfg.D, cfg.De, cfg.FD

    x1_d = nc.dram_tensor("x1", [HB, 128, Wc, D], BF16, kind="ExternalInput")
    x2_d = nc.dram_tensor("x2", [HB + 2, 128, Wc + 2, De], BF16,
                          kind="ExternalInput")
    sel_d = nc.dram_tensor("selmats", [128, 128], BF16, kind="ExternalInput")
    w0a_d = nc.dram_tensor("w0a", [128, MID], FP32, kind="ExternalInput")
    w0b_d = nc.dram_tensor("w0b", [88, MID], FP32, kind="ExternalInput")
    w1a_d = nc.dram_tensor("w1ra", [MID, 128], FP32, kind="ExternalInput")
    w1b_d = nc.dram_tensor("w1rb", [MID, 88], FP32, kind="ExternalInput")
    b0_d = nc.dram_tensor("b0c", [MID, 1], FP32, kind="ExternalInput")
    b1a_d = nc.dram_tensor("b1ra", [128, 1], FP32, kind="ExternalInput")
    b1b_d = nc.dram_tensor("b1rb", [88, 1], FP32, kind="ExternalInput")
    out_d = nc.dram_tensor("out", [HB, 216, FD], BF16, kind="ExternalOutput")

    n_drain = HB * cfg.n_fs
    # CC1 fires mid-way through row `jh`'s products and covers rows 0..jh-1;
    # leaving ~3 rows (~50us) after it absorbs cross-core jitter + CC latency
    # well before CC2's input is ready.
    jh = max(1, HB - 4)
    early = jh * cfg.n_fs           # accum cols covered by CC1

    with tile.TileContext(nc) as tc:
        with (
            tc.tile_pool(name="const", bufs=1) as cpool,
            tc.tile_pool(name="ps", bufs=1, space="PSUM") as ps,
            tc.tile_pool(name="dram", bufs=1, space="DRAM") as dram,
            tc.tile_pool(name="pp", bufs=6) as ppool,
        ):
            spool = ppool
            # resident tiles
            x1t = cpool.tile([128, HB, Wc, D], BF16)
            x2t = cpool.tile([128, HB + 2, Wc + 2, De], BF16)
            corrA = cpool.tile([128, HB, FD], BF16)
            corrB = cpool.tile([88, HB, FD], BF16)
            selt = cpool.tile([128, 128], BF16)
            w0at = cpool.tile([128, MID], FP32)
            w0bt = cpool.tile([88, MID], FP32)
            w1at = cpool.tile([MID, 128], FP32)
            w1bt = cpool.tile([MID, 88], FP32)
            b0t = cpool.tile([MID, 1], FP32)
            b1at = cpool.tile([128, 1], FP32)
            b1bt = cpool.tile([88, 1], FP32)
            accA = cpool.tile([128, n_drain], FP32)
            accB = cpool.tile([88, n_drain], FP32)
            # Collective staging: col 0 = A partials (128 rows), col 1 = B
            # partials (88 rows + zero pad). One [128,2] DMA each way per
            # CC instead of two per side.
            ccw_in = cpool.tile([128, 2], FP32)
            cc1s = cpool.tile([128, 2], FP32)
            cc2s = cpool.tile([128, 2], FP32)
            cc1_out = cpool.tile([128, 2], FP32)
            cc2_out = cpool.tile([128, 2], FP32)
            dumin = cpool.tile([1, 2], FP32)
            dumout = cpool.tile([1, 2], FP32)
            ccw_in_d = dram.tile([128, 2], FP32)
            ccw_out_d = dram.tile([128, 2], FP32)
            cc1_in_d = dram.tile([128, 2], FP32)
            cc1_out_d = dram.tile([128, 2], FP32)
            cc2_in_d = dram.tile([128, 2], FP32)
            cc2_out_d = dram.tile([128, 2], FP32)

            rr_state = [0]
            rr_engs = [nc.gpsimd, nc.sync, nc.scalar]

            def _rr():
                e = rr_engs[rr_state[0] % len(rr_engs)]
                rr_state[0] += 1
                return e

            def load_x1_row(r, parts=1, eng=None):
                cuts = [round(i * Wc / parts) for i in range(parts + 1)]
                for a, b in zip(cuts, cuts[1:]):
                    if a < b:
                        (eng or _rr()).dma_start(x1t[:, r, a:b, :],
                                                 x1_d[r, :, a:b, :])

            def load_x2_row(r, parts=1, eng=None):
                cuts = [round(i * (Wc + 2) / parts) for i in range(parts + 1)]
                for a, b in zip(cuts, cuts[1:]):
                    if a < b:
                        (eng or _rr()).dma_start(x2t[:, r, a:b, :],
                                                 x2_d[r, :, a:b, :])

            # priority ramp: first product (dx=-1,dy=-1) needs x2 row 0 +
            # x1 row 0. Two big halves each on the sync+gpsimd queues;
            # scalar preloads the sigmoid ACT table (tail then pays no
            # table swap) and loads selt, then joins the DMA round-robin.
            wh = (Wc + 2) // 2
            xh = max(Wc // 2, 1)
            nc.sync.dma_start(x2t[:, 0, 0:wh, :], x2_d[0, :, 0:wh, :])
            nc.gpsimd.dma_start(x2t[:, 0, wh:Wc + 2, :],
                                x2_d[0, :, wh:Wc + 2, :])
            nc.scalar.dma_start(x1t[:, 0, 0:xh, :], x1_d[0, :, 0:xh, :])
            nc.sync.dma_start(x1t[:, 0, xh:Wc, :], x1_d[0, :, xh:Wc, :])
            nc.gpsimd.memset(dumin[:], 0.0)
            nc.gpsimd.memset(ccw_in[:], 0.0)
            nc.gpsimd.memset(cc1s[:, 1:2], 0.0)
            nc.gpsimd.memset(cc2s[:, 1:2], 0.0)
            if HB == 1:
                nc.gpsimd.memset(cc1_out[:], 0.0)
            nc.scalar.activation(dumout[:], dumin[:], AF.Sigmoid)
            nc.gpsimd.dma_start(selt[:], sel_d[:])
            rr_state[0] = 0
            load_x2_row(1, parts=3)
            load_x2_row(2, parts=3)
            if HB > 1:
                load_x1_row(1, parts=3)
            load_x2_row(3, parts=2)
            nc.sync.dma_start(w0at[:], w0a_d[:])
            nc.sync.dma_start(w0bt[:], w0b_d[:])
            nc.sync.dma_start(w1at[:], w1a_d[:])
            nc.sync.dma_start(w1bt[:], w1b_d[:])
            nc.sync.dma_start(b0t[:], b0_d[:])
            nc.sync.dma_start(b1at[:], b1a_d[:])
            nc.sync.dma_start(b1bt[:], b1b_d[:])

            # Warm-up collective: absorbs cross-core launch skew and CC
            # firmware setup so the real allreduces only pay marginal cost.
            nc.gpsimd.dma_start(ccw_in_d[:], ccw_in[:])
            nc.gpsimd.collective_compute(
                "AllReduce", ALU.add,
                replica_groups=[list(range(N_CORES))],
                ins=[ccw_in_d[:].opt()],
                outs=[ccw_out_d[:].opt()],
            )

            # remaining loads, ~one row ahead of first use
            nx1 = 2
            for r in range(4, HB + 2):
                if nx1 < min(r - 1, HB):
                    load_x1_row(nx1)
                    nx1 += 1
                load_x2_row(r)
            while nx1 < HB:
                load_x1_row(nx1)
                nx1 += 1

            a_tot = {g: 4 for g in range(4)}
            b_tot = {0: 4, 1: 4, 2: 3}

            def emit_products(j, mid_hook=None):
                """9 fused (sx,sy) product ops for row j; each computes all
                three dz shifts via an overlapping unit-stride AP."""
                prods = {}
                for t, (dx, dy) in enumerate(
                        (a, b) for a in range(3) for b in range(3)):
                    if t == 4 and mid_hook is not None:
                        mid_hook()
                    pt = ppool.tile([128, 3, FD], BF16, tag="P", bufs=6)
                    base = x2t[:, j + dy, dx:dx + Wc, 0:De]
                    u = base.unsqueeze(1)
                    ap2 = [list(p) for p in u.ap]
                    ap2[1] = [1, 3]
                    ap2[3] = [1, D]
                    srcv = _AP(tensor=u.tensor, offset=u.offset, ap=ap2)
                    x1b = x1t[:, j].unsqueeze(1).broadcast_to([128, 3, Wc, D])
                    dst = pt[:].rearrange("p r (w d) -> p r w d", d=D)
                    nc.vector.tensor_tensor(dst, x1b, srcv, ALU.mult)
                    kb = 9 * dx + 3 * dy
                    for dz in range(3):
                        prods[kb + dz] = (pt, dz)
                return prods

            def emit_reduce_row(j, prods):
                """PE reduction + ACT drains for row j, natural k order."""
                psA = [ps.tile([128, hi - lo], FP32, tag=f"psA{i}",
                               name=f"psA{i}", padded_shape=[128, 512])
                       for i, (lo, hi) in enumerate(cfg.slices)]
                psB = [ps.tile([128, hi - lo], FP32, tag=f"psB{i}",
                               name=f"psB{i}", padded_shape=[128, 512])
                       for i, (lo, hi) in enumerate(cfg.slices)]
                seen = {}
                for k in range(K):
                    is_a, g, v = _gv_of(k)
                    tot = a_tot[g] if is_a else b_tot[g]
                    cnt = seen.get((is_a, g), 0)
                    seen[(is_a, g)] = cnt + 1
                    pst = psA if is_a else psB
                    pk, idx = prods[k]
                    for i, (lo, hi) in enumerate(cfg.slices):
                        nc.tensor.matmul(
                            pst[i][32 * g:32 * g + 32, :],
                            selt[:, 32 * v:32 * v + 32],
                            pk[:, idx, lo:hi],
                            start=(cnt == 0), stop=(cnt == tot - 1),
                            tile_position=(0, 32 * g),
                        )
                    if k == 15:
                        # A chains complete: drain now so the next row's
                        # A matmuls never wait on these banks.
                        for i, (lo, hi) in enumerate(cfg.slices):
                            di = j * cfg.n_fs + i
                            nc.scalar.activation(
                                corrA[:, j, lo:hi], psA[i][:], AF.Copy,
                                accum_out=accA[:, di:di + 1])
                for i, (lo, hi) in enumerate(cfg.slices):
                    di = j * cfg.n_fs + i
                    nc.scalar.activation(
                        corrB[:, j, lo:hi], psB[i][0:88, :], AF.Copy,
                        accum_out=accB[:, di:di + 1])

            def cc1_hook():
                # rows 0..jh-1 partial sums: reduce on DVE straight into the
                # SBUF cc buffer, allreduce while products still run (frees
                # the CC stream long before CC2's input is ready)
                nc.vector.tensor_reduce(cc1s[:, 0:1], accA[:, 0:early],
                                        mybir.AxisListType.X, ALU.add)
                nc.vector.tensor_reduce(cc1s[0:88, 1:2], accB[:, 0:early],
                                        mybir.AxisListType.X, ALU.add)
                nc.gpsimd.dma_start(cc1_in_d[:], cc1s[:])
                nc.gpsimd.collective_compute(
                    "AllReduce", ALU.add,
                    replica_groups=[list(range(N_CORES))],
                    ins=[cc1_in_d[:].opt()],
                    outs=[cc1_out_d[:].opt()],
                )
                nc.gpsimd.dma_start(cc1_out[:], cc1_out_d[:])

            for j in range(HB):
                hook = cc1_hook if (j == jh and HB > 1) else None
                prods = emit_products(j, mid_hook=hook)
                emit_reduce_row(j, prods)

            # ---- last rows' partials -> CC2 (small, warm, SBUF) ----
            nc.vector.tensor_reduce(cc2s[:, 0:1], accA[:, early:n_drain],
                                    mybir.AxisListType.X, ALU.add)
            nc.vector.tensor_reduce(cc2s[0:88, 1:2], accB[:, early:n_drain],
                                    mybir.AxisListType.X, ALU.add)
            nc.gpsimd.dma_start(cc2_in_d[:], cc2s[:])
            nc.gpsimd.collective_compute(
                "AllReduce", ALU.add,
                replica_groups=[list(range(N_CORES))],
                ins=[cc2_in_d[:].opt()],
                outs=[cc2_out_d[:].opt()],
            )
            nc.sync.dma_start(cc2_out[:], cc2_out_d[:])

            # ---- gate MLP ----
            hps = ps.tile([MID, 1], FP32, tag="psA0", padded_shape=[128, 512])
            nc.tensor.matmul(hps[:], w0at[:], cc1_out[:, 0:1],
                             start=True, stop=False)
            nc.tensor.matmul(hps[:], w0bt[:], cc1_out[0:88, 1:2],
                             start=False, stop=False)
            nc.tensor.matmul(hps[:], w0at[:], cc2_out[:, 0:1],
                             start=False, stop=False)
            nc.tensor.matmul(hps[:], w0bt[:], cc2_out[0:88, 1:2],
                             start=False, stop=True)
            h0 = cpool.tile([MID, 1], FP32)
            hvec = cpool.tile([MID, 1], FP32)
            nc.vector.tensor_tensor(h0[:], hps[:], b0t[:], ALU.add)
            nc.vector.tensor_scalar(hvec[:], h0[:], 0.0, None, ALU.max)
            gpsA = ps.tile([128, 1], FP32, tag="psA1", padded_shape=[128, 512])
            gpsB = ps.tile([88, 1], FP32, tag="psA2", padded_shape=[128, 512])
            nc.tensor.matmul(gpsA[:], w1at[:], hvec[:], start=True, stop=True)
            nc.tensor.matmul(gpsB[:], w1bt[:], hvec[:], start=True, stop=True)
            gA = cpool.tile([128, 1], FP32)
            gB = cpool.tile([88, 1], FP32)
            nc.scalar.activation(gA[:], gpsA[:], AF.Sigmoid, bias=b1at[:],
                                 scale=1.0)
            nc.scalar.activation(gB[:], gpsB[:], AF.Sigmoid, bias=b1bt[:],
                                 scale=1.0)

            # ---- gated writeout. Gating on ACT (1/3 of A rows) + DVE;
            # full-row DMAs on HWDGE queues only (sync + scalar). ----
            for j in range(HB):
                stA = spool.tile([128, FD], BF16, tag="gsA", bufs=6)
                if j % 3 == 0:
                    nc.scalar.mul(stA[:], corrA[:, j, :], gA[:])
                else:
                    nc.vector.tensor_scalar(stA[:], corrA[:, j, :], gA[:],
                                            None, ALU.mult)
                stB = spool.tile([88, FD], BF16, tag="gsB", bufs=6)
                nc.vector.tensor_scalar(stB[:], corrB[:, j, :], gB[:],
                                        None, ALU.mult)
                nc.sync.dma_start(out_d[j, 0:128, :], stA[:])
                (nc.sync if j % 2 == 0 else nc.scalar).dma_start(
                    out_d[j, 128:216, :], stB[:])

    nc.compile()
    return nc


# ---------------- host-side prep / assembly ----------------

def make_gate_consts(w0, b0, w1, b1, cfg: Cfg):
    norm = 1.0 / (cfg.W * cfg.H * cfg.D)
    sel = np.zeros((128, 128), dtype=np.float32)
    for v in range(4):
        for c in range(C):
            for h8 in range(H8):
                sel[c * H8 + h8, 32 * v + 8 * v + h8] = 1.0 / 16
    w0 = np.asarray(w0, dtype=np.float32)
    w1 = np.asarray(w1, dtype=np.float32)
    b1 = np.asarray(b1, dtype=np.float32)
    w0a = np.zeros((128, MID), dtype=np.float32)
    w0b = np.zeros((88, MID), dtype=np.float32)
    w1ra = np.zeros((MID, 128), dtype=np.float32)
    w1rb = np.zeros((MID, 88), dtype=np.float32)
    b1ra = np.zeros((128, 1), dtype=np.float32)
    b1rb = np.zeros((88, 1), dtype=np.float32)
    for k in range(K):
        for h8 in range(H8):
            r = _row_of(k, h8)
            if k < 16:
                w0a[r, :] = w0[:, k] * norm
                w1ra[:, r] = w1[k, :]
                b1ra[r, 0] = b1[k]
            else:
                w0b[r - 128, :] = w0[:, k] * norm
                w1rb[:, r - 128] = w1[k, :]
                b1rb[r - 128, 0] = b1[k]
    return {
        "selmats": sel.astype(ml_dtypes.bfloat16),
        "w0a": w0a, "w0b": w0b, "w1ra": w1ra, "w1rb": w1rb,
        "b0c": np.asarray(b0, dtype=np.float32).reshape(MID, 1),
        "b1ra": b1ra, "b1rb": b1rb,
    }


def _fold(a, HB):
    # [C, w, H, D'] -> [(c h8), hblk, w, d]
    Cc, ww, hh, dd = a.shape
    a = a.reshape(Cc, ww, H8, HB, dd)
    a = np.ascontiguousarray(a.transpose(0, 2, 3, 1, 4))
    return a.reshape(C * H8, HB, ww, dd)


def make_inputs_per_core(x_1, x_2, w0, b0, w1, b1, cfg: Cfg):
    """x_1/x_2: [1, C, W, H, D] float32 -> list of per-core input dicts."""
    W, H, D, De = cfg.W, cfg.H, cfg.D, cfg.De
    Wc, HB = cfg.Wc, cfg.HB
    x1 = np.asarray(x_1)[0].astype(ml_dtypes.bfloat16)      # [C, W, H, D]
    x2 = np.asarray(x_2)[0].astype(ml_dtypes.bfloat16)
    # padded x2: w +-1, h +-1, d in [-1, D+1)
    x2p = np.zeros((C, W + 2, H + 2, D + 2), dtype=ml_dtypes.bfloat16)
    x2p[:, 1:W + 1, 1:H + 1, 1:D + 1] = x2
    # hblk-extended h indices: row r of (h8) block = x2p h-index h8*HB + r,
    # covering h = h8*HB - 1 .. (h8+1)*HB (1-voxel halo on both sides)
    hidx = (np.arange(H8) * HB)[:, None] + np.arange(HB + 2)  # [H8, HB+2]

    consts = make_gate_consts(w0, b0, w1, b1, cfg)
    in_maps = []
    for ci in range(N_CORES):
        ws = ci * Wc
        m = dict(consts)
        m["x1"] = np.ascontiguousarray(
            _fold(x1[:, ws:ws + Wc, :, :], HB).transpose(1, 0, 2, 3))
        blk = x2p[:, ws:ws + Wc + 2, :, :]                  # [C, Wc+2, H+2, De]
        oo = blk[:, :, hidx, 0:De]                          # [C, Wc+2, H8, HB+2, De]
        m["x2"] = np.ascontiguousarray(
            oo.transpose(3, 0, 2, 1, 4)).reshape(HB + 2, 128, Wc + 2, De)
        in_maps.append(m)
    return in_maps


def assemble_output(results, cfg: Cfg):
    W, H, D = cfg.W, cfg.H, cfg.D
    Wc, HB = cfg.Wc, cfg.HB
    rows = np.empty((K, H8), dtype=np.int64)
    for k in range(K):
        for h8 in range(H8):
            rows[k, h8] = _row_of(k, h8)
    out = np.empty((W, H, D, K), dtype=np.float32)
    for ci, r in enumerate(results):
        o = np.asarray(r["out"]).reshape(HB, 216, Wc, D)
        o = o.transpose(1, 0, 2, 3)
        core = o[rows]                        # [K, H8, HB, Wc, D]
        core = core.transpose(3, 1, 2, 4, 0)  # [Wc, H8, HB, D, K]
        out[ci * Wc:(ci + 1) * Wc] = core.reshape(Wc, H, D, K)
    return out[None]


_CACHE = {}
TRACE = False           # test harness can set kernel.TRACE = True


def kernel(x_1, x_2, w0, b0, w1, b1):
    cfg = Cfg()
    if "nc" not in _CACHE:
        _CACHE["nc"] = build_nc(cfg)
    nc = _CACHE["nc"]
    in_maps = make_inputs_per_core(x_1, x_2, w0, b0, w1, b1, cfg)
    last_exc = None
    for _attempt in range(3):
        try:
            res = run_bass_kernel_spmd(nc, in_maps,
                                       core_ids=list(range(N_CORES)),
                                       trace=TRACE)
            break
        except Exception as e:  # transient NRT device errors: retry
            last_exc = e
    else:
        raise last_exc
    _CACHE["last_res"] = res
    return assemble_output(res.results, cfg)


# revision 23
# speedup vs baseline: 1.1782x; 1.0009x over previous
"""Trainium2 Bass kernel for shifted-window correlation (27 shifts) + SE gate.

Reference computation (shapes hardcoded; B=1, C=16, W=80, H=96, D=112):
  corr[w,h,d,k] = mean_c x1[c,w,h,d] * x2[c, w+sx, h+sy, d+sz]   (zero-padded)
  s = mean_{w,h,d} corr;  g = sigmoid(w1 @ relu(w0 @ s + b0) + b1)
  out = corr * g

Strategy (8 cores, W sharded 10/core), v2:
  - SBUF partition dim = (c:16, h8:8) where h8 = h // (H/8).
  - ONE resident x2 tile per core ([128, HB+2, Wc+2, De], De=D+2) whose
    hblk axis carries a 1-row halo; all 27 shifts are free-dim offsets.
    (bf16 DVE speed is offset-alignment-independent — measured — so no
    even/odd d copies.)
  - Products on DVE (bf16 2x mode, ~0.52ns/elem): one fused op per
    (sx,sy) computing all three dz via an overlapping unit-stride AP
    (3 free dims is the TensorTensor ISA limit) -> 9 ops/row.
  - Channel reduction on PE via a block-diagonal selection matmul
    packing (k,h8) into 128/88-row PSUM tiles; natural k order so
    consecutive matmuls cycle the 3 slice banks; A banks drain on ACT
    (capturing squeeze partials via accum_out) right after k=15 so the
    next row never stalls on PSUM.
  - Squeeze allreduce split in two: CC1 ([216,1], rows 0..HB-2) fires
    mid-way through the last row's products and absorbs cross-core
    skew; CC2 ([216,1], last row) right after the final drain only
    pays warm-latency (~9us). Gate MLP accumulates both columns.
  - relu on DVE + sigmoid table preloaded at ramp so the tail has no
    ACT_TABLE_LOAD.
  - Gated writeout overlaps: ACT gates 1/3 of A rows + DVE the rest;
    full-row output DMAs on HWDGE queues only (sync + a few on scalar)
    -- gpsimd SWDGE would starve against DVE perf-mode gating ops.
"""

import sys
import types

import numpy as np
import ml_dtypes


def _install_ntff_hook_shim():
    """agent image's antenv lacks axon_hooks; needed only for trace=True."""
    if "antenv.axon_hooks" in sys.modules:
        return
    try:
        import antenv
        from trn_agent_boot.trn_boot import _ntff_profile_via_ctypes

        hook = _ntff_profile_via_ctypes("/opt/axon/libaxon_pjrt.so")
        mod = types.ModuleType("antenv.axon_hooks")
        ref = {"h": hook}
        mod.get_axon_ntff_profile_hook = lambda: ref["h"]
        mod.set_axon_ntff_profile_hook = lambda h: ref.__setitem__("h", h)
        sys.modules["antenv.axon_hooks"] = mod
        antenv.axon_hooks = mod
    except Exception:
        pass


_install_ntff_hook_shim()

import concourse.bacc as bacc  # noqa: E402
import concourse.tile as tile  # noqa: E402
import concourse.mybir as mybir  # noqa: E402
from concourse.ap import AP as _AP  # noqa: E402
from concourse.bass_utils import run_bass_kernel_spmd  # noqa: E402

BF16 = mybir.dt.bfloat16
FP32 = mybir.dt.float32
AF = mybir.ActivationFunctionType
ALU = mybir.AluOpType

N_CORES = 8
C = 16
H8 = 8          # partition sub-dim over h
K = 27
MID = 6


class Cfg:
    def __init__(self, W=80, H=96, D=112):
        assert H % H8 == 0
        self.W, self.H, self.D = W, H, D
        self.Wc = W // N_CORES          # w columns per core
        self.HB = H // H8               # hblk extent (free dim)
        self.De = D + 2                 # padded d extent
        self.FD = self.Wc * D           # flat (w, d) free size per row
        self.slices = [(o, min(o + 512, self.FD))
                       for o in range(0, self.FD, 512)]
        self.n_fs = len(self.slices)


# shift order matches reference: k = dx*9 + dy*3 + dz, s* = d*-1
SHIFTS = [(dx - 1, dy - 1, dz - 1)
          for dx in range(3) for dy in range(3) for dz in range(3)]


def _gv_of(k):
    """(is_A, psum column group, selection slice) for shift k."""
    kk = k if k < 16 else k - 16
    return k < 16, kk // 4, kk % 4


def _row_of(k, h8):
    """corr partition row for (k, h8). Tile A: k 0..15, tile B: 16..26."""
    kk = k if k < 16 else k - 16
    base = 0 if k < 16 else 128
    return base + 32 * (kk // 4) + 8 * (kk % 4) + h8


def build_nc(cfg: Cfg):
    nc = bacc.Bacc("TRN2", target_bir_lowering=False, debug=False,
                   num_devices=N_CORES)
    HB, Wc, D, De, FD = cfg.HB, cfg.Wc, cfg.D, cfg.De, cfg.FD

    x1_d = nc.dram_tensor("x1", [HB, 128, Wc, D], BF16, kind="ExternalInput")
    x2_d = nc.dram_tensor("x2", [HB + 2, 128, Wc + 2, De], BF16,
                          kind="ExternalInput")
    sel_d = nc.dram_tensor("selmats", [128, 128], BF16, kind="ExternalInput")
    w0a_d = nc.dram_tensor("w0a", [128, MID], FP32, kind="ExternalInput")
    w0b_d = nc.dram_tensor("w0b", [88, MID], FP32, kind="ExternalInput")
    w1a_d = nc.dram_tensor("w1ra", [MID, 128], FP32, kind="ExternalInput")
    w1b_d = nc.dram_tensor("w1rb", [MID, 88], FP32, kind="ExternalInput")
    b0_d = nc.dram_tensor("b0c", [MID, 1], FP32, kind="ExternalInput")
    b1a_d = nc.dram_tensor("b1ra", [128, 1], FP32, kind="ExternalInput")
    b1b_d = nc.dram_tensor("b1rb", [88, 1], FP32, kind="ExternalInput")
    out_d = nc.dram_tensor("out", [HB, 216, FD], BF16, kind="ExternalOutput")

    n_drain = HB * cfg.n_fs
    # CC1 fires mid-way through row `jh`'s products and covers rows 0..jh-1;
    # leaving ~3 rows (~50us) after it absorbs cross-core jitter + CC latency
    # well before CC2's input is ready.
    jh = max(1, HB - 4)
    early = jh * cfg.n_fs           # accum cols covered by CC1

    with tile.TileContext(nc) as tc:
        with (
            tc.tile_pool(name="const", bufs=1) as cpool,
            tc.tile_pool(name="ps", bufs=1, space="PSUM") as ps,
            tc.tile_pool(name="dram", bufs=1, space="DRAM") as dram,
            tc.tile_pool(name="pp", bufs=6) as ppool,
        ):
            spool = ppool
            # resident tiles
            x1t = cpool.tile([128, HB, Wc, D], BF16)
            x2t = cpool.tile([128, HB + 2, Wc + 2, De], BF16)
            corrA = cpool.tile([128, HB, FD], BF16)
            corrB = cpool.tile([88, HB, FD], BF16)
            selt = cpool.tile([128, 128], BF16)
            w0at = cpool.tile([128, MID], FP32)
            w0bt = cpool.tile([88, MID], FP32)
            w1at = cpool.tile([MID, 128], FP32)
            w1bt = cpool.tile([MID, 88], FP32)
            b0t = cpool.tile([MID, 1], FP32)
            b1at = cpool.tile([128, 1], FP32)
            b1bt = cpool.tile([88, 1], FP32)
            accA = cpool.tile([128, n_drain], FP32)
            accB = cpool.tile([88, n_drain], FP32)
            # Collective staging: col 0 = A partials (128 rows), col 1 = B
            # partials (88 rows + zero pad). One [128,2] DMA each way per
            # CC instead of two per side.
            ccw_in = cpool.tile([128, 2], FP32)
            cc1s = cpool.tile([128, 2], FP32)
            cc2s = cpool.tile([128, 2], FP32)
            cc1_out = cpool.tile([128, 2], FP32)
            cc2_out = cpool.tile([128, 2], FP32)
            dumin = cpool.tile([1, 2], FP32)
            dumout = cpool.tile([1, 2], FP32)
            ccw_in_d = dram.tile([128, 2], FP32)
            ccw_out_d = dram.tile([128, 2], FP32)
            cc1_in_d = dram.tile([128, 2], FP32)
            cc1_out_d = dram.tile([128, 2], FP32)
            cc2_in_d = dram.tile([128, 2], FP32)
            cc2_out_d = dram.tile([128, 2], FP32)

            rr_state = [0]
            rr_engs = [nc.gpsimd, nc.sync, nc.scalar]

            def _rr():
                e = rr_engs[rr_state[0] % len(rr_engs)]
                rr_state[0] += 1
                return e

            def load_x1_row(r, parts=1, eng=None):
                cuts = [round(i * Wc / parts) for i in range(parts + 1)]
                for a, b in zip(cuts, cuts[1:]):
                    if a < b:
                        (eng or _rr()).dma_start(x1t[:, r, a:b, :],
                                                 x1_d[r, :, a:b, :])

            def load_x2_row(r, parts=1, eng=None):
                cuts = [round(i * (Wc + 2) / parts) for i in range(parts + 1)]
                for a, b in zip(cuts, cuts[1:]):
                    if a < b:
                        (eng or _rr()).dma_start(x2t[:, r, a:b, :],
                                                 x2_d[r, :, a:b, :])

            # priority ramp: first product (dx=-1,dy=-1) needs x2 row 0 +
            # x1 row 0. Two big halves each on the sync+gpsimd queues;
            # scalar preloads the sigmoid ACT table (tail then pays no
            # table swap) and loads selt, then joins the DMA round-robin.
            wh = (Wc + 2) // 2
            xh = max(Wc // 2, 1)
            nc.sync.dma_start(x2t[:, 0, 0:wh, :], x2_d[0, :, 0:wh, :])
            nc.gpsimd.dma_start(x2t[:, 0, wh:Wc + 2, :],
                                x2_d[0, :, wh:Wc + 2, :])
            nc.scalar.dma_start(x1t[:, 0, 0:xh, :], x1_d[0, :, 0:xh, :])
            nc.sync.dma_start(x1t[:, 0, xh:Wc, :], x1_d[0, :, xh:Wc, :])
            nc.gpsimd.memset(dumin[:], 0.0)
            nc.gpsimd.memset(ccw_in[:], 0.0)
            nc.gpsimd.memset(cc1s[:, 1:2], 0.0)
            nc.gpsimd.memset(cc2s[:, 1:2], 0.0)
            if HB == 1:
                nc.gpsimd.memset(cc1_out[:], 0.0)
            nc.scalar.activation(dumout[:], dumin[:], AF.Sigmoid)
            nc.gpsimd.dma_start(selt[:], sel_d[:])
            rr_state[0] = 0
            load_x2_row(1, parts=3)
            load_x2_row(2, parts=3)
            if HB > 1:
                load_x1_row(1, parts=3)
            load_x2_row(3, parts=2)
            nc.sync.dma_start(w0at[:], w0a_d[:])
            nc.sync.dma_start(w0bt[:], w0b_d[:])
            nc.sync.dma_start(w1at[:], w1a_d[:])
            nc.sync.dma_start(w1bt[:], w1b_d[:])
            nc.sync.dma_start(b0t[:], b0_d[:])
            nc.sync.dma_start(b1at[:], b1a_d[:])
            nc.sync.dma_start(b1bt[:], b1b_d[:])

            # Warm-up collective: absorbs cross-core launch skew and CC
            # firmware setup so the real allreduces only pay marginal cost.
            nc.gpsimd.dma_start(ccw_in_d[:], ccw_in[:])
            nc.gpsimd.collective_compute(
                "AllReduce", ALU.add,
                replica_groups=[list(range(N_CORES))],
                ins=[ccw_in_d[:].opt()],
                outs=[ccw_out_d[:].opt()],
            )

            # remaining loads, ~one row ahead of first use
            nx1 = 2
            for r in range(4, HB + 2):
                if nx1 < min(r - 1, HB):
                    load_x1_row(nx1)
                    nx1 += 1
                load_x2_row(r)
            while nx1 < HB:
                load_x1_row(nx1)
                nx1 += 1

            a_tot = {g: 4 for g in range(4)}
            b_tot = {0: 4, 1: 4, 2: 3}

            def emit_products(j, mid_hook=None):
                """9 fused (sx,sy) product ops for row j; each computes all
                three dz shifts via an overlapping unit-stride AP."""
                prods = {}
                for t, (dx, dy) in enumerate(
                        (a, b) for a in range(3) for b in range(3)):
                    if t == 4 and mid_hook is not None:
                        mid_hook()
                    pt = ppool.tile([128, 3, FD], BF16, tag="P", bufs=6)
                    base = x2t[:, j + dy, dx:dx + Wc, 0:De]
                    u = base.unsqueeze(1)
                    ap2 = [list(p) for p in u.ap]
                    ap2[1] = [1, 3]
                    ap2[3] = [1, D]
                    srcv = _AP(tensor=u.tensor, offset=u.offset, ap=ap2)
                    x1b = x1t[:, j].unsqueeze(1).broadcast_to([128, 3, Wc, D])
                    dst = pt[:].rearrange("p r (w d) -> p r w d", d=D)
                    nc.vector.tensor_tensor(dst, x1b, srcv, ALU.mult)
                    kb = 9 * dx + 3 * dy
                    for dz in range(3):
                        prods[kb + dz] = (pt, dz)
                return prods

            def emit_reduce_row(j, prods):
                """PE reduction + ACT drains for row j, natural k order."""
                psA = [ps.tile([128, hi - lo], FP32, tag=f"psA{i}",
                               name=f"psA{i}", padded_shape=[128, 512])
                       for i, (lo, hi) in enumerate(cfg.slices)]
                psB = [ps.tile([128, hi - lo], FP32, tag=f"psB{i}",
                               name=f"psB{i}", padded_shape=[128, 512])
                       for i, (lo, hi) in enumerate(cfg.slices)]
                seen = {}
                for k in range(K):
                    is_a, g, v = _gv_of(k)
                    tot = a_tot[g] if is_a else b_tot[g]
                    cnt = seen.get((is_a, g), 0)
                    seen[(is_a, g)] = cnt + 1
                    pst = psA if is_a else psB
                    pk, idx = prods[k]
                    for i, (lo, hi) in enumerate(cfg.slices):
                        nc.tensor.matmul(
                            pst[i][32 * g:32 * g + 32, :],
                            selt[:, 32 * v:32 * v + 32],
                            pk[:, idx, lo:hi],
                            start=(cnt == 0), stop=(cnt == tot - 1),
                            tile_position=(0, 32 * g),
                        )
                    if k == 15:
                        # A chains complete: drain now so the next row's
                        # A matmuls never wait on these banks.
                        for i, (lo, hi) in enumerate(cfg.slices):
                            di = j * cfg.n_fs + i
                            nc.scalar.activation(
                                corrA[:, j, lo:hi], psA[i][:], AF.Copy,
                                accum_out=accA[:, di:di + 1])
                for i, (lo, hi) in enumerate(cfg.slices):
                    di = j * cfg.n_fs + i
                    nc.scalar.activation(
                        corrB[:, j, lo:hi], psB[i][0:88, :], AF.Copy,
                        accum_out=accB[:, di:di + 1])

            def cc1_hook():
                # rows 0..jh-1 partial sums: reduce on DVE straight into the
                # SBUF cc buffer, allreduce while products still run (frees
                # the CC stream long before CC2's input is ready)
                nc.vector.tensor_reduce(cc1s[:, 0:1], accA[:, 0:early],
                                        mybir.AxisListType.X, ALU.add)
                nc.vector.tensor_reduce(cc1s[0:88, 1:2], accB[:, 0:early],
                                        mybir.AxisListType.X, ALU.add)
                nc.gpsimd.dma_start(cc1_in_d[:], cc1s[:])
                nc.gpsimd.collective_compute(
                    "AllReduce", ALU.add,
                    replica_groups=[list(range(N_CORES))],
                    ins=[cc1_in_d[:].opt()],
                    outs=[cc1_out_d[:].opt()],
                )
                nc.gpsimd.dma_start(cc1_out[:], cc1_out_d[:])

            for j in range(HB):
                hook = cc1_hook if (j == jh and HB > 1) else None
                prods = emit_products(j, mid_hook=hook)
                emit_reduce_row(j, prods)

            # ---- last rows' partials -> CC2 (small, warm, SBUF) ----
            nc.vector.tensor_reduce(cc2s[:, 0:1], accA[:, early:n_drain],
                                    mybir.AxisListType.X, ALU.add)
            nc.vector.tensor_reduce(cc2s[0:88, 1:2], accB[:, early:n_drain],
                                    mybir.AxisListType.X, ALU.add)
            nc.sync.dma_start(cc2_in_d[:], cc2s[:])
            nc.gpsimd.collective_compute(
                "AllReduce", ALU.add,
                replica_groups=[list(range(N_CORES))],
                ins=[cc2_in_d[:].opt()],
                outs=[cc2_out_d[:].opt()],
            )
            nc.sync.dma_start(cc2_out[:], cc2_out_d[:])

            # ---- gate MLP ----
            hps = ps.tile([MID, 1], FP32, tag="psA0", padded_shape=[128, 512])
            nc.tensor.matmul(hps[:], w0at[:], cc1_out[:, 0:1],
                             start=True, stop=False)
            nc.tensor.matmul(hps[:], w0bt[:], cc1_out[0:88, 1:2],
                             start=False, stop=False)
            nc.tensor.matmul(hps[:], w0at[:], cc2_out[:, 0:1],
                             start=False, stop=False)
            nc.tensor.matmul(hps[:], w0bt[:], cc2_out[0:88, 1:2],
                             start=False, stop=True)
            h0 = cpool.tile([MID, 1], FP32)
            hvec = cpool.tile([MID, 1], FP32)
            nc.vector.tensor_tensor(h0[:], hps[:], b0t[:], ALU.add)
            nc.vector.tensor_scalar(hvec[:], h0[:], 0.0, None, ALU.max)
            gpsA = ps.tile([128, 1], FP32, tag="psA1", padded_shape=[128, 512])
            gpsB = ps.tile([88, 1], FP32, tag="psA2", padded_shape=[128, 512])
            nc.tensor.matmul(gpsA[:], w1at[:], hvec[:], start=True, stop=True)
            nc.tensor.matmul(gpsB[:], w1bt[:], hvec[:], start=True, stop=True)
            gA = cpool.tile([128, 1], FP32)
            gB = cpool.tile([88, 1], FP32)
            nc.scalar.activation(gA[:], gpsA[:], AF.Sigmoid, bias=b1at[:],
                                 scale=1.0)
            nc.scalar.activation(gB[:], gpsB[:], AF.Sigmoid, bias=b1bt[:],
                                 scale=1.0)

            # ---- gated writeout. Gating on ACT (1/3 of A rows) + DVE;
            # full-row DMAs on HWDGE queues only (sync + scalar). ----
            for j in range(HB):
                stA = spool.tile([128, FD], BF16, tag="gsA", bufs=6)
                if j % 3 == 0:
                    nc.scalar.mul(stA[:], corrA[:, j, :], gA[:])
                else:
                    nc.vector.tensor_scalar(stA[:], corrA[:, j, :], gA[:],
                                            None, ALU.mult)
                stB = spool.tile([88, FD], BF16, tag="gsB", bufs=6)
                nc.vector.tensor_scalar(stB[:], corrB[:, j, :], gB[:],
                                        None, ALU.mult)
                nc.sync.dma_start(out_d[j, 0:128, :], stA[:])
                nc.sync.dma_start(out_d[j, 128:216, :], stB[:])

    nc.compile()
    return nc


# ---------------- host-side prep / assembly ----------------

def make_gate_consts(w0, b0, w1, b1, cfg: Cfg):
    norm = 1.0 / (cfg.W * cfg.H * cfg.D)
    sel = np.zeros((128, 128), dtype=np.float32)
    for v in range(4):
        for c in range(C):
            for h8 in range(H8):
                sel[c * H8 + h8, 32 * v + 8 * v + h8] = 1.0 / 16
    w0 = np.asarray(w0, dtype=np.float32)
    w1 = np.asarray(w1, dtype=np.float32)
    b1 = np.asarray(b1, dtype=np.float32)
    w0a = np.zeros((128, MID), dtype=np.float32)
    w0b = np.zeros((88, MID), dtype=np.float32)
    w1ra = np.zeros((MID, 128), dtype=np.float32)
    w1rb = np.zeros((MID, 88), dtype=np.float32)
    b1ra = np.zeros((128, 1), dtype=np.float32)
    b1rb = np.zeros((88, 1), dtype=np.float32)
    for k in range(K):
        for h8 in range(H8):
            r = _row_of(k, h8)
            if k < 16:
                w0a[r, :] = w0[:, k] * norm
                w1ra[:, r] = w1[k, :]
                b1ra[r, 0] = b1[k]
            else:
                w0b[r - 128, :] = w0[:, k] * norm
                w1rb[:, r - 128] = w1[k, :]
                b1rb[r - 128, 0] = b1[k]
    return {
        "selmats": sel.astype(ml_dtypes.bfloat16),
        "w0a": w0a, "w0b": w0b, "w1ra": w1ra, "w1rb": w1rb,
        "b0c": np.asarray(b0, dtype=np.float32).reshape(MID, 1),
        "b1ra": b1ra, "b1rb": b1rb,
    }


def _fold(a, HB):
    # [C, w, H, D'] -> [(c h8), hblk, w, d]
    Cc, ww, hh, dd = a.shape
    a = a.reshape(Cc, ww, H8, HB, dd)
    a = np.ascontiguousarray(a.transpose(0, 2, 3, 1, 4))
    return a.reshape(C * H8, HB, ww, dd)


def make_inputs_per_core(x_1, x_2, w0, b0, w1, b1, cfg: Cfg):
    """x_1/x_2: [1, C, W, H, D] float32 -> list of per-core input dicts."""
    W, H, D, De = cfg.W, cfg.H, cfg.D, cfg.De
    Wc, HB = cfg.Wc, cfg.HB
    x1 = np.asarray(x_1)[0].astype(ml_dtypes.bfloat16)      # [C, W, H, D]
    x2 = np.asarray(x_2)[0].astype(ml_dtypes.bfloat16)
    # padded x2: w +-1, h +-1, d in [-1, D+1)
    x2p = np.zeros((C, W + 2, H + 2, D + 2), dtype=ml_dtypes.bfloat16)
    x2p[:, 1:W + 1, 1:H + 1, 1:D + 1] = x2
    # hblk-extended h indices: row r of (h8) block = x2p h-index h8*HB + r,
    # covering h = h8*HB - 1 .. (h8+1)*HB (1-voxel halo on both sides)
    hidx = (np.arange(H8) * HB)[:, None] + np.arange(HB + 2)  # [H8, HB+2]

    consts = make_gate_consts(w0, b0, w1, b1, cfg)
    in_maps = []
    for ci in range(N_CORES):
        ws = ci * Wc
        m = dict(consts)
        m["x1"] = np.ascontiguousarray(
            _fold(x1[:, ws:ws + Wc, :, :], HB).transpose(1, 0, 2, 3))
        blk = x2p[:, ws:ws + Wc + 2, :, :]                  # [C, Wc+2, H+2, De]
        oo = blk[:, :, hidx, 0:De]                          # [C, Wc+2, H8, HB+2, De]
        m["x2"] = np.ascontiguousarray(
            oo.transpose(3, 0, 2, 1, 4)).reshape(HB + 2, 128, Wc + 2, De)
        in_maps.append(m)
    return in_maps


def assemble_output(results, cfg: Cfg):
    W, H, D = cfg.W, cfg.H, cfg.D
    Wc, HB = cfg.Wc, cfg.HB
    rows = np.empty((K, H8), dtype=np.int64)
    for k in range(K):
        for h8 in range(H8):
            rows[k, h8] = _row_of(k, h8)
    out = np.empty((W, H, D, K), dtype=np.float32)
    for ci, r in enumerate(results):
        o = np.asarray(r["out"]).reshape(HB, 216, Wc, D)
        o = o.transpose(1, 0, 2, 3)
        core = o[rows]                        # [K, H8, HB, Wc, D]
        core = core.transpose(3, 1, 2, 4, 0)  # [Wc, H8, HB, D, K]
        out[ci * Wc:(ci + 1) * Wc] = core.reshape(Wc, H, D, K)
    return out[None]


_CACHE = {}
TRACE = False           # test harness can set kernel.TRACE = True


def kernel(x_1, x_2, w0, b0, w1, b1):
    cfg = Cfg()
    if "nc" not in _CACHE:
        _CACHE["nc"] = build_nc(cfg)
    nc = _CACHE["nc"]
    in_maps = make_inputs_per_core(x_1, x_2, w0, b0, w1, b1, cfg)
    last_exc = None
    for _attempt in range(3):
        try:
            res = run_bass_kernel_spmd(nc, in_maps,
                                       core_ids=list(range(N_CORES)),
                                       trace=TRACE)
            break
        except Exception as e:  # transient NRT device errors: retry
            last_exc = e
    else:
        raise last_exc
    _CACHE["last_res"] = res
    return assemble_output(res.results, cfg)


# revision 25
# speedup vs baseline: 1.1852x; 1.0060x over previous
"""Trainium2 Bass kernel for shifted-window correlation (27 shifts) + SE gate.

Reference computation (shapes hardcoded; B=1, C=16, W=80, H=96, D=112):
  corr[w,h,d,k] = mean_c x1[c,w,h,d] * x2[c, w+sx, h+sy, d+sz]   (zero-padded)
  s = mean_{w,h,d} corr;  g = sigmoid(w1 @ relu(w0 @ s + b0) + b1)
  out = corr * g

Strategy (8 cores, W sharded 10/core), v2:
  - SBUF partition dim = (c:16, h8:8) where h8 = h // (H/8).
  - ONE resident x2 tile per core ([128, HB+2, Wc+2, De], De=D+2) whose
    hblk axis carries a 1-row halo; all 27 shifts are free-dim offsets.
    (bf16 DVE speed is offset-alignment-independent — measured — so no
    even/odd d copies.)
  - Products on DVE (bf16 2x mode, ~0.52ns/elem): one fused op per
    (sx,sy) computing all three dz via an overlapping unit-stride AP
    (3 free dims is the TensorTensor ISA limit) -> 9 ops/row.
  - Channel reduction on PE via a block-diagonal selection matmul
    packing (k,h8) into 128/88-row PSUM tiles; natural k order so
    consecutive matmuls cycle the 3 slice banks; A banks drain on ACT
    (capturing squeeze partials via accum_out) right after k=15 so the
    next row never stalls on PSUM.
  - Squeeze allreduce split in two: CC1 ([216,1], rows 0..HB-2) fires
    mid-way through the last row's products and absorbs cross-core
    skew; CC2 ([216,1], last row) right after the final drain only
    pays warm-latency (~9us). Gate MLP accumulates both columns.
  - relu on DVE + sigmoid table preloaded at ramp so the tail has no
    ACT_TABLE_LOAD.
  - Gated writeout overlaps: ACT gates 1/3 of A rows + DVE the rest;
    full-row output DMAs on HWDGE queues only (sync + a few on scalar)
    -- gpsimd SWDGE would starve against DVE perf-mode gating ops.
"""

import sys
import types

import numpy as np
import ml_dtypes


def _install_ntff_hook_shim():
    """agent image's antenv lacks axon_hooks; needed only for trace=True."""
    if "antenv.axon_hooks" in sys.modules:
        return
    try:
        import antenv
        from trn_agent_boot.trn_boot import _ntff_profile_via_ctypes

        hook = _ntff_profile_via_ctypes("/opt/axon/libaxon_pjrt.so")
        mod = types.ModuleType("antenv.axon_hooks")
        ref = {"h": hook}
        mod.get_axon_ntff_profile_hook = lambda: ref["h"]
        mod.set_axon_ntff_profile_hook = lambda h: ref.__setitem__("h", h)
        sys.modules["antenv.axon_hooks"] = mod
        antenv.axon_hooks = mod
    except Exception:
        pass


_install_ntff_hook_shim()

import concourse.bacc as bacc  # noqa: E402
import concourse.tile as tile  # noqa: E402
import concourse.mybir as mybir  # noqa: E402
from concourse.ap import AP as _AP  # noqa: E402
from concourse.bass_utils import run_bass_kernel_spmd  # noqa: E402

BF16 = mybir.dt.bfloat16
FP32 = mybir.dt.float32
AF = mybir.ActivationFunctionType
ALU = mybir.AluOpType

N_CORES = 8
C = 16
H8 = 8          # partition sub-dim over h
K = 27
MID = 6


class Cfg:
    def __init__(self, W=80, H=96, D=112):
        assert H % H8 == 0
        self.W, self.H, self.D = W, H, D
        self.Wc = W // N_CORES          # w columns per core
        self.HB = H // H8               # hblk extent (free dim)
        self.De = D + 2                 # padded d extent
        self.FD = self.Wc * D           # flat (w, d) free size per row
        self.slices = [(o, min(o + 512, self.FD))
                       for o in range(0, self.FD, 512)]
        self.n_fs = len(self.slices)


# shift order matches reference: k = dx*9 + dy*3 + dz, s* = d*-1
SHIFTS = [(dx - 1, dy - 1, dz - 1)
          for dx in range(3) for dy in range(3) for dz in range(3)]


def _gv_of(k):
    """(is_A, psum column group, selection slice) for shift k."""
    kk = k if k < 16 else k - 16
    return k < 16, kk // 4, kk % 4


def _row_of(k, h8):
    """corr partition row for (k, h8). Tile A: k 0..15, tile B: 16..26."""
    kk = k if k < 16 else k - 16
    base = 0 if k < 16 else 128
    return base + 32 * (kk // 4) + 8 * (kk % 4) + h8


def build_nc(cfg: Cfg):
    nc = bacc.Bacc("TRN2", target_bir_lowering=False, debug=False,
                   num_devices=N_CORES)
    HB, Wc, D, De, FD = cfg.HB, cfg.Wc, cfg.D, cfg.De, cfg.FD

    x1_d = nc.dram_tensor("x1", [HB, 128, Wc, D], BF16, kind="ExternalInput")
    x2_d = nc.dram_tensor("x2", [HB + 2, 128, Wc + 2, De], BF16,
                          kind="ExternalInput")
    sel_d = nc.dram_tensor("selmats", [128, 128], BF16, kind="ExternalInput")
    w0a_d = nc.dram_tensor("w0a", [128, MID], FP32, kind="ExternalInput")
    w0b_d = nc.dram_tensor("w0b", [88, MID], FP32, kind="ExternalInput")
    w1a_d = nc.dram_tensor("w1ra", [MID, 128], FP32, kind="ExternalInput")
    w1b_d = nc.dram_tensor("w1rb", [MID, 88], FP32, kind="ExternalInput")
    b0_d = nc.dram_tensor("b0c", [MID, 1], FP32, kind="ExternalInput")
    b1a_d = nc.dram_tensor("b1ra", [128, 1], FP32, kind="ExternalInput")
    b1b_d = nc.dram_tensor("b1rb", [88, 1], FP32, kind="ExternalInput")
    out_d = nc.dram_tensor("out", [HB, 216, FD], BF16, kind="ExternalOutput")

    n_drain = HB * cfg.n_fs
    # CC1 fires mid-way through row `jh`'s products and covers rows 0..jh-1;
    # leaving ~3 rows (~50us) after it absorbs cross-core jitter + CC latency
    # well before CC2's input is ready.
    jh = max(1, HB - 4)
    early = jh * cfg.n_fs           # accum cols covered by CC1

    with tile.TileContext(nc) as tc:
        with (
            tc.tile_pool(name="const", bufs=1) as cpool,
            tc.tile_pool(name="ps", bufs=1, space="PSUM") as ps,
            tc.tile_pool(name="dram", bufs=1, space="DRAM") as dram,
            tc.tile_pool(name="pp", bufs=6) as ppool,
        ):
            spool = ppool
            # resident tiles
            x1t = cpool.tile([128, HB, Wc, D], BF16)
            x2t = cpool.tile([128, HB + 2, Wc + 2, De], BF16)
            corrA = cpool.tile([128, HB, FD], BF16)
            corrB = cpool.tile([88, HB, FD], BF16)
            selt = cpool.tile([128, 128], BF16)
            w0at = cpool.tile([128, MID], FP32)
            w0bt = cpool.tile([88, MID], FP32)
            w1at = cpool.tile([MID, 128], FP32)
            w1bt = cpool.tile([MID, 88], FP32)
            b0t = cpool.tile([MID, 1], FP32)
            b1at = cpool.tile([128, 1], FP32)
            b1bt = cpool.tile([88, 1], FP32)
            accA = cpool.tile([128, n_drain], FP32)
            accB = cpool.tile([88, n_drain], FP32)
            # Collective staging: col 0 = A partials (128 rows), col 1 = B
            # partials (88 rows + zero pad). One [128,2] DMA each way per
            # CC instead of two per side.
            ccw_in = cpool.tile([128, 2], FP32)
            cc1s = cpool.tile([128, 2], FP32)
            cc2s = cpool.tile([128, 2], FP32)
            cc1_out = cpool.tile([128, 2], FP32)
            cc2_out = cpool.tile([128, 2], FP32)
            dumin = cpool.tile([1, 2], FP32)
            dumout = cpool.tile([1, 2], FP32)
            ccw_in_d = dram.tile([128, 2], FP32)
            ccw_out_d = dram.tile([128, 2], FP32)
            cc1_in_d = dram.tile([128, 2], FP32)
            cc1_out_d = dram.tile([128, 2], FP32)
            cc2_in_d = dram.tile([128, 2], FP32)
            cc2_out_d = dram.tile([128, 2], FP32)

            rr_state = [0]
            rr_engs = [nc.gpsimd, nc.sync, nc.scalar]

            def _rr():
                e = rr_engs[rr_state[0] % len(rr_engs)]
                rr_state[0] += 1
                return e

            def load_x1_row(r, parts=1, eng=None):
                cuts = [round(i * Wc / parts) for i in range(parts + 1)]
                for a, b in zip(cuts, cuts[1:]):
                    if a < b:
                        (eng or _rr()).dma_start(x1t[:, r, a:b, :],
                                                 x1_d[r, :, a:b, :])

            def load_x2_row(r, parts=1, eng=None):
                cuts = [round(i * (Wc + 2) / parts) for i in range(parts + 1)]
                for a, b in zip(cuts, cuts[1:]):
                    if a < b:
                        (eng or _rr()).dma_start(x2t[:, r, a:b, :],
                                                 x2_d[r, :, a:b, :])

            # priority ramp: first product (dx=-1,dy=-1) needs x2 row 0 +
            # x1 row 0. Two big halves each on the sync+gpsimd queues;
            # scalar preloads the sigmoid ACT table (tail then pays no
            # table swap) and loads selt, then joins the DMA round-robin.
            wh = (Wc + 2) // 2
            xh = max(Wc // 2, 1)
            nc.sync.dma_start(x2t[:, 0, 0:wh, :], x2_d[0, :, 0:wh, :])
            nc.gpsimd.dma_start(x2t[:, 0, wh:Wc + 2, :],
                                x2_d[0, :, wh:Wc + 2, :])
            nc.scalar.dma_start(x1t[:, 0, 0:xh, :], x1_d[0, :, 0:xh, :])
            nc.sync.dma_start(x1t[:, 0, xh:Wc, :], x1_d[0, :, xh:Wc, :])
            nc.gpsimd.memset(dumin[:], 0.0)
            nc.gpsimd.memset(ccw_in[:], 0.0)
            nc.gpsimd.memset(cc1s[:, 1:2], 0.0)
            nc.gpsimd.memset(cc2s[:, 1:2], 0.0)
            if HB == 1:
                nc.gpsimd.memset(cc1_out[:], 0.0)
            nc.scalar.activation(dumout[:], dumin[:], AF.Sigmoid)
            nc.gpsimd.dma_start(selt[:], sel_d[:])
            rr_state[0] = 0
            load_x2_row(1, parts=3)
            load_x2_row(2, parts=3)
            if HB > 1:
                load_x1_row(1, parts=3)
            load_x2_row(3, parts=2)
            nc.sync.dma_start(w0at[:], w0a_d[:])
            nc.sync.dma_start(w0bt[:], w0b_d[:])
            nc.sync.dma_start(w1at[:], w1a_d[:])
            nc.sync.dma_start(w1bt[:], w1b_d[:])
            nc.sync.dma_start(b0t[:], b0_d[:])
            nc.sync.dma_start(b1at[:], b1a_d[:])
            nc.sync.dma_start(b1bt[:], b1b_d[:])

            # Warm-up collective: absorbs cross-core launch skew and CC
            # firmware setup so the real allreduces only pay marginal cost.
            nc.gpsimd.dma_start(ccw_in_d[:], ccw_in[:])
            nc.gpsimd.collective_compute(
                "AllReduce", ALU.add,
                replica_groups=[list(range(N_CORES))],
                ins=[ccw_in_d[:].opt()],
                outs=[ccw_out_d[:].opt()],
            )

            # remaining loads, ~one row ahead of first use
            nx1 = 2
            for r in range(4, HB + 2):
                if nx1 < min(r - 1, HB):
                    load_x1_row(nx1)
                    nx1 += 1
                load_x2_row(r)
            while nx1 < HB:
                load_x1_row(nx1)
                nx1 += 1

            a_tot = {g: 4 for g in range(4)}
            b_tot = {0: 4, 1: 4, 2: 3}

            def emit_products(j, mid_hook=None):
                """9 fused (sx,sy) product ops for row j; each computes all
                three dz shifts via an overlapping unit-stride AP."""
                prods = {}
                for t, (dx, dy) in enumerate(
                        (a, b) for a in range(3) for b in range(3)):
                    if t == 4 and mid_hook is not None:
                        mid_hook()
                    pt = ppool.tile([128, 3, FD], BF16, tag="P", bufs=6)
                    base = x2t[:, j + dy, dx:dx + Wc, 0:De]
                    u = base.unsqueeze(1)
                    ap2 = [list(p) for p in u.ap]
                    ap2[1] = [1, 3]
                    ap2[3] = [1, D]
                    srcv = _AP(tensor=u.tensor, offset=u.offset, ap=ap2)
                    x1b = x1t[:, j].unsqueeze(1).broadcast_to([128, 3, Wc, D])
                    dst = pt[:].rearrange("p r (w d) -> p r w d", d=D)
                    nc.vector.tensor_tensor(dst, x1b, srcv, ALU.mult)
                    kb = 9 * dx + 3 * dy
                    for dz in range(3):
                        prods[kb + dz] = (pt, dz)
                return prods

            def emit_reduce_row(j, prods, last=False):
                """PE reduction + ACT drains for row j, natural k order.
                On the last row the B drains skip accum_out (the squeeze
                contribution is folded straight from PSUM on the DVE) so
                ACT's serialized accumulator reads leave the CC2 path."""
                psA = [ps.tile([128, hi - lo], FP32, tag=f"psA{i}",
                               name=f"psA{i}", padded_shape=[128, 512])
                       for i, (lo, hi) in enumerate(cfg.slices)]
                psB = [ps.tile([128, hi - lo], FP32, tag=f"psB{i}",
                               name=f"psB{i}", padded_shape=[128, 512])
                       for i, (lo, hi) in enumerate(cfg.slices)]
                seen = {}
                for k in range(K):
                    is_a, g, v = _gv_of(k)
                    tot = a_tot[g] if is_a else b_tot[g]
                    cnt = seen.get((is_a, g), 0)
                    seen[(is_a, g)] = cnt + 1
                    pst = psA if is_a else psB
                    pk, idx = prods[k]
                    for i, (lo, hi) in enumerate(cfg.slices):
                        nc.tensor.matmul(
                            pst[i][32 * g:32 * g + 32, :],
                            selt[:, 32 * v:32 * v + 32],
                            pk[:, idx, lo:hi],
                            start=(cnt == 0), stop=(cnt == tot - 1),
                            tile_position=(0, 32 * g),
                        )
                    if k == 15:
                        # A chains complete: drain now so the next row's
                        # A matmuls never wait on these banks.
                        for i, (lo, hi) in enumerate(cfg.slices):
                            di = j * cfg.n_fs + i
                            nc.scalar.activation(
                                corrA[:, j, lo:hi], psA[i][:], AF.Copy,
                                accum_out=accA[:, di:di + 1])
                for i, (lo, hi) in enumerate(cfg.slices):
                    di = j * cfg.n_fs + i
                    if last:
                        nc.scalar.activation(corrB[:, j, lo:hi],
                                             psB[i][0:88, :], AF.Copy)
                    else:
                        nc.scalar.activation(
                            corrB[:, j, lo:hi], psB[i][0:88, :], AF.Copy,
                            accum_out=accB[:, di:di + 1])
                return psB

            def cc1_hook():
                # rows 0..jh-1 partial sums: reduce on DVE straight into the
                # SBUF cc buffer, allreduce while products still run (frees
                # the CC stream long before CC2's input is ready)
                nc.vector.tensor_reduce(cc1s[:, 0:1], accA[:, 0:early],
                                        mybir.AxisListType.X, ALU.add)
                nc.vector.tensor_reduce(cc1s[0:88, 1:2], accB[:, 0:early],
                                        mybir.AxisListType.X, ALU.add)
                nc.gpsimd.dma_start(cc1_in_d[:], cc1s[:])
                nc.gpsimd.collective_compute(
                    "AllReduce", ALU.add,
                    replica_groups=[list(range(N_CORES))],
                    ins=[cc1_in_d[:].opt()],
                    outs=[cc1_out_d[:].opt()],
                )
                nc.gpsimd.dma_start(cc1_out[:], cc1_out_d[:])

            psB_last = None
            for j in range(HB):
                hook = cc1_hook if (j == jh and HB > 1) else None
                prods = emit_products(j, mid_hook=hook)
                psB_last = emit_reduce_row(j, prods, last=(j == HB - 1))

            # ---- last rows' partials -> CC2 (small, warm, aligned).
            # A partials are final at the last row's k=15 drain; ship that
            # column while the B side folds from PSUM on the idle DVE. ----
            nc.vector.tensor_reduce(cc2s[:, 0:1], accA[:, early:n_drain],
                                    mybir.AxisListType.X, ALU.add)
            nc.sync.dma_start(cc2_in_d[:, 0:1], cc2s[:, 0:1])
            ptmpB = cpool.tile([88, cfg.n_fs + 1], FP32)
            for i in range(cfg.n_fs):
                nc.vector.tensor_reduce(ptmpB[:, i:i + 1],
                                        psB_last[i][0:88, :],
                                        mybir.AxisListType.X, ALU.add)
            if early < n_drain - cfg.n_fs:
                nc.vector.tensor_reduce(ptmpB[:, cfg.n_fs:cfg.n_fs + 1],
                                        accB[:, early:n_drain - cfg.n_fs],
                                        mybir.AxisListType.X, ALU.add)
            else:
                nc.gpsimd.memset(ptmpB[:, cfg.n_fs:cfg.n_fs + 1], 0.0)
            nc.vector.tensor_reduce(cc2s[0:88, 1:2], ptmpB[:],
                                    mybir.AxisListType.X, ALU.add)
            nc.sync.dma_start(cc2_in_d[:, 1:2], cc2s[:, 1:2])
            nc.gpsimd.collective_compute(
                "AllReduce", ALU.add,
                replica_groups=[list(range(N_CORES))],
                ins=[cc2_in_d[:].opt()],
                outs=[cc2_out_d[:].opt()],
            )
            nc.sync.dma_start(cc2_out[:], cc2_out_d[:])

            # ---- gate MLP ----
            hps = ps.tile([MID, 1], FP32, tag="psA0", padded_shape=[128, 512])
            nc.tensor.matmul(hps[:], w0at[:], cc1_out[:, 0:1],
                             start=True, stop=False)
            nc.tensor.matmul(hps[:], w0bt[:], cc1_out[0:88, 1:2],
                             start=False, stop=False)
            nc.tensor.matmul(hps[:], w0at[:], cc2_out[:, 0:1],
                             start=False, stop=False)
            nc.tensor.matmul(hps[:], w0bt[:], cc2_out[0:88, 1:2],
                             start=False, stop=True)
            h0 = cpool.tile([MID, 1], FP32)
            hvec = cpool.tile([MID, 1], FP32)
            nc.vector.tensor_tensor(h0[:], hps[:], b0t[:], ALU.add)
            nc.vector.tensor_scalar(hvec[:], h0[:], 0.0, None, ALU.max)
            gpsA = ps.tile([128, 1], FP32, tag="psA1", padded_shape=[128, 512])
            gpsB = ps.tile([88, 1], FP32, tag="psA2", padded_shape=[128, 512])
            nc.tensor.matmul(gpsA[:], w1at[:], hvec[:], start=True, stop=True)
            nc.tensor.matmul(gpsB[:], w1bt[:], hvec[:], start=True, stop=True)
            gA = cpool.tile([128, 1], FP32)
            gB = cpool.tile([88, 1], FP32)
            nc.scalar.activation(gA[:], gpsA[:], AF.Sigmoid, bias=b1at[:],
                                 scale=1.0)
            nc.scalar.activation(gB[:], gpsB[:], AF.Sigmoid, bias=b1bt[:],
                                 scale=1.0)

            # ---- gated writeout. Gating on ACT (1/3 of A rows) + DVE;
            # full-row DMAs on HWDGE queues only (sync + scalar). ----
            for j in range(HB):
                stA = spool.tile([128, FD], BF16, tag="gsA", bufs=6)
                if j % 3 == 0:
                    nc.scalar.mul(stA[:], corrA[:, j, :], gA[:])
                else:
                    nc.vector.tensor_scalar(stA[:], corrA[:, j, :], gA[:],
                                            None, ALU.mult)
                stB = spool.tile([88, FD], BF16, tag="gsB", bufs=6)
                nc.vector.tensor_scalar(stB[:], corrB[:, j, :], gB[:],
                                        None, ALU.mult)
                nc.sync.dma_start(out_d[j, 0:128, :], stA[:])
                nc.sync.dma_start(out_d[j, 128:216, :], stB[:])

    nc.compile()
    return nc


# ---------------- host-side prep / assembly ----------------

def make_gate_consts(w0, b0, w1, b1, cfg: Cfg):
    norm = 1.0 / (cfg.W * cfg.H * cfg.D)
    sel = np.zeros((128, 128), dtype=np.float32)
    for v in range(4):
        for c in range(C):
            for h8 in range(H8):
                sel[c * H8 + h8, 32 * v + 8 * v + h8] = 1.0 / 16
    w0 = np.asarray(w0, dtype=np.float32)
    w1 = np.asarray(w1, dtype=np.float32)
    b1 = np.asarray(b1, dtype=np.float32)
    w0a = np.zeros((128, MID), dtype=np.float32)
    w0b = np.zeros((88, MID), dtype=np.float32)
    w1ra = np.zeros((MID, 128), dtype=np.float32)
    w1rb = np.zeros((MID, 88), dtype=np.float32)
    b1ra = np.zeros((128, 1), dtype=np.float32)
    b1rb = np.zeros((88, 1), dtype=np.float32)
    for k in range(K):
        for h8 in range(H8):
            r = _row_of(k, h8)
            if k < 16:
                w0a[r, :] = w0[:, k] * norm
                w1ra[:, r] = w1[k, :]
                b1ra[r, 0] = b1[k]
            else:
                w0b[r - 128, :] = w0[:, k] * norm
                w1rb[:, r - 128] = w1[k, :]
                b1rb[r - 128, 0] = b1[k]
    return {
        "selmats": sel.astype(ml_dtypes.bfloat16),
        "w0a": w0a, "w0b": w0b, "w1ra": w1ra, "w1rb": w1rb,
        "b0c": np.asarray(b0, dtype=np.float32).reshape(MID, 1),
        "b1ra": b1ra, "b1rb": b1rb,
    }


def _fold(a, HB):
    # [C, w, H, D'] -> [(c h8), hblk, w, d]
    Cc, ww, hh, dd = a.shape
    a = a.reshape(Cc, ww, H8, HB, dd)
    a = np.ascontiguousarray(a.transpose(0, 2, 3, 1, 4))
    return a.reshape(C * H8, HB, ww, dd)


def make_inputs_per_core(x_1, x_2, w0, b0, w1, b1, cfg: Cfg):
    """x_1/x_2: [1, C, W, H, D] float32 -> list of per-core input dicts."""
    W, H, D, De = cfg.W, cfg.H, cfg.D, cfg.De
    Wc, HB = cfg.Wc, cfg.HB
    x1 = np.asarray(x_1)[0].astype(ml_dtypes.bfloat16)      # [C, W, H, D]
    x2 = np.asarray(x_2)[0].astype(ml_dtypes.bfloat16)
    # padded x2: w +-1, h +-1, d in [-1, D+1)
    x2p = np.zeros((C, W + 2, H + 2, D + 2), dtype=ml_dtypes.bfloat16)
    x2p[:, 1:W + 1, 1:H + 1, 1:D + 1] = x2
    # hblk-extended h indices: row r of (h8) block = x2p h-index h8*HB + r,
    # covering h = h8*HB - 1 .. (h8+1)*HB (1-voxel halo on both sides)
    hidx = (np.arange(H8) * HB)[:, None] + np.arange(HB + 2)  # [H8, HB+2]

    consts = make_gate_consts(w0, b0, w1, b1, cfg)
    in_maps = []
    for ci in range(N_CORES):
        ws = ci * Wc
        m = dict(consts)
        m["x1"] = np.ascontiguousarray(
            _fold(x1[:, ws:ws + Wc, :, :], HB).transpose(1, 0, 2, 3))
        blk = x2p[:, ws:ws + Wc + 2, :, :]                  # [C, Wc+2, H+2, De]
        oo = blk[:, :, hidx, 0:De]                          # [C, Wc+2, H8, HB+2, De]
        m["x2"] = np.ascontiguousarray(
            oo.transpose(3, 0, 2, 1, 4)).reshape(HB + 2, 128, Wc + 2, De)
        in_maps.append(m)
    return in_maps


def assemble_output(results, cfg: Cfg):
    W, H, D = cfg.W, cfg.H, cfg.D
    Wc, HB = cfg.Wc, cfg.HB
    rows = np.empty((K, H8), dtype=np.int64)
    for k in range(K):
        for h8 in range(H8):
            rows[k, h8] = _row_of(k, h8)
    out = np.empty((W, H, D, K), dtype=np.float32)
    for ci, r in enumerate(results):
        o = np.asarray(r["out"]).reshape(HB, 216, Wc, D)
        o = o.transpose(1, 0, 2, 3)
        core = o[rows]                        # [K, H8, HB, Wc, D]
        core = core.transpose(3, 1, 2, 4, 0)  # [Wc, H8, HB, D, K]
        out[ci * Wc:(ci + 1) * Wc] = core.reshape(Wc, H, D, K)
    return out[None]


_CACHE = {}
TRACE = False           # test harness can set kernel.TRACE = True


def kernel(x_1, x_2, w0, b0, w1, b1):
    cfg = Cfg()
    if "nc" not in _CACHE:
        _CACHE["nc"] = build_nc(cfg)
    nc = _CACHE["nc"]
    in_maps = make_inputs_per_core(x_1, x_2, w0, b0, w1, b1, cfg)
    last_exc = None
    for _attempt in range(3):
        try:
            res = run_bass_kernel_spmd(nc, in_maps,
                                       core_ids=list(range(N_CORES)),
                                       trace=TRACE)
            break
        except Exception as e:  # transient NRT device errors: retry
            last_exc = e
    else:
        raise last_exc
    _CACHE["last_res"] = res
    return assemble_output(res.results, cfg)
